# revision 19
# baseline (speedup 1.0000x reference)
"""Trainium2 Bass kernel for nn_BiLSTM_M_61615600828569 (segment_reduce).

Full computation per batch:
  span_emb = masked-max-pool of token windows   (B,256,768)
  vertex_emb = masked-mean over coref spans     (B,128,768)
  head/tail  = vertex gather by relation        (B,512,768)
  feat = [head, eh, tail, et, head*tail]        (B,512,2344)
  out  = relu(feat @ W1) @ W2 + b2              (B,512,97)

Sharding: data-parallel over batch; 16 batches / 8 cores = 2 per core.
All index work is precomputed on host; float math runs on device in bf16
with fp32 PSUM accumulation, in transposed layout (features on
partitions) so the final predict.T has the 97 classes on partitions for
a per-partition bias add.

Span pooling: spans are sorted by width per batch (the permutation is
folded into the host-built pool matrix, so it is free).  The widest 128
spans ("cc0", width>=3 whp) are fetched as two exact-cover 4-row quads
(start and start+w-3); the narrowest 128 ("cc1", width<=4 whp) as three
exact-cover 2-row pairs.  Exact cover means the max tree needs no row
masks (duplicated rows are harmless under max), except one leaf mask
for width-0 spans; mask-free levels run as single strided DVE ops since
DVE cost is dominated by a ~1us per-instruction overhead.  A rare
width distribution that breaks the cc0/cc1 bounds (~4-sigma) falls back
to a masked 8-row variant.

DMA schedule: only small tables load up front so the span gathers own
the HBM bandwidth; the big W1/W2 load is released by a manual semaphore
bumped by batch-0's first gather completion.  dis_embed@W1 blocks are
folded on host into one 40-row contraction; V_emb.T comes from PE
transposes packed four-to-a-PSUM-bank.
"""
import numpy as np
import ml_dtypes
from contextlib import ExitStack

import concourse.bass as bass
import concourse.bacc as bacc
import concourse.tile as tile
from concourse import mybir
from concourse import bass_utils

BF16 = ml_dtypes.bfloat16

B, S, D = 16, 1024, 768
NS, MAXW = 256, 8
V, C = 128, 6
R = 512
REL, HID, DIS = 97, 384, 20
NEG = -2e30

NCORES = 8
NB = B // NCORES          # batches per core = 2
NEGROW = NB * S           # two -inf rows appended for width-0 pair redirects
SENT_ROWS = NB * S + 2
NW1C = 18                 # W1 chunks: a(6) + c(6) + e(6)
W2C = HID // 128          # 3
GCOLS = 40                # gidx cols per batch: 16 (quads) + 24 (pairs)

# bf16 "small" pack layout (columns)
PK_ID = 0                 # identity [128,128]
PK_PT = 128               # poolt    [128, NB*2*V=512]
PK_EW = 640               # ew2      [40, 384]
PK_ES = 1024              # esel2    [40, NB*R=1024]
PK_N = 2048
# f32 pack layout (columns)
FK_MASK = 0               # masks [128, NB*2*7]
FK_INV = 28               # invcnt [128, NB]
FK_B2 = 30                # b2 [97, 1]
FK_N = 31
# w1/w2 pack (bf16) columns
WK_W1 = 0                 # [128, 18*384]
WK_W2 = NW1C * 384        # [128, 3*97]
WK_N = WK_W2 + W2C * REL


def _patch_drain_and_barrier():
    """Walrus rejects >1 explicit sync wait on a Drain (TPB_CTRL), but Tile's
    tail drain waits on every used proc sem at once. Emit one single-wait
    drain per proc instead; the final drain then needs no waits."""
    import concourse.tile as tile_mod
    from concourse.vector_clock import VectorClock, ScopedClock

    if getattr(tile_mod.TileContext, "_ant_drain_patched", False):
        return

    def _patched(self, tick_clock, wait_clock):
        full = tick_clock.global_clock
        n = len(full)
        engines = [self.nc.sync, self.nc.vector, self.nc.scalar,
                   self.nc.tensor, self.nc.gpsimd]
        for i, p in enumerate([q for q in range(n) if full[q] > 0]):
            vec = [full[q] if q == p else 0 for q in range(n)]
            d = engines[i % len(engines)].drain()
            wait_clock.add_sem_waits(d.ins, ScopedClock({None: VectorClock(vec)}))
        self.nc.sync.drain()
        self.nc.all_engine_barrier()
        popped = self.nc._tile_sem_poison_stack.pop()
        assert popped is self._sem_poison
        self.nc.clear_and_free_semaphores(list(self.sems.allocated().values()))
        self.nc.all_engine_barrier()

    tile_mod.TileContext._drain_and_barrier = _patched
    tile_mod.TileContext._ant_drain_patched = True


_patch_drain_and_barrier()

_NC_CACHE = {}


def _build(variant):
    """One-core program; SPMD-replicated across the 8 cores.
    variant: 'exact' (cc0 w>=3, cc1 w<=4) or 'masked88' fallback."""
    bf = mybir.dt.bfloat16
    f32 = mybir.dt.float32
    AF = mybir.ActivationFunctionType
    ADD = mybir.AluOpType.add
    MAX = mybir.AluOpType.max
    MUL = mybir.AluOpType.mult

    nc = bacc.Bacc("TRN2", target_bir_lowering=False, debug=False, num_devices=1)

    sent = nc.dram_tensor("sent", (SENT_ROWS, D), bf, kind="ExternalInput")
    gidx = nc.dram_tensor("gidx", (128, NB * GCOLS), mybir.dt.int16, kind="ExternalInput")
    pkf = nc.dram_tensor("pkf", (128, FK_N), f32, kind="ExternalInput")
    pkb = nc.dram_tensor("pkb", (128, PK_N), bf, kind="ExternalInput")
    hts = nc.dram_tensor("hts", (V, 2 * NB * R), bf, kind="ExternalInput")
    wpk = nc.dram_tensor("wpk", (128, WK_N), bf, kind="ExternalInput")
    outd = nc.dram_tensor("outd", (REL, NB, R), bf, kind="ExternalOutput")

    def sent_view(nrows):
        return bass.AP(tensor=sent.ap().tensor, offset=0,
                       ap=[[D, SENT_ROWS - (nrows - 1)], [1, nrows * D]])

    with tile.TileContext(nc) as tc, ExitStack() as ctx:
        consts = ctx.enter_context(tc.tile_pool(name="consts", bufs=1))
        work = ctx.enter_context(tc.tile_pool(name="work", bufs=1))
        perb = ctx.enter_context(tc.tile_pool(name="perb", bufs=2))
        psums = ctx.enter_context(tc.tile_pool(name="psums", bufs=1, space="PSUM"))

        def psum_tile(name, tag, bufs):
            return psums.tile([128, R], mybir.dt.float32, space="PSUM",
                              tag=tag, bufs=bufs, name=name)

        # ---- gather index table from GpSimd (its own drain covers the dep);
        #      other small tables from sync
        idx_t = consts.tile([128, NB * GCOLS], mybir.dt.int16)
        nc.gpsimd.dma_start(out=idx_t[:], in_=gidx.ap())
        pkf_t = consts.tile([128, FK_N], f32)
        nc.sync.dma_start(out=pkf_t[:], in_=pkf.ap())
        pkb_t = consts.tile([128, PK_N], bf)
        nc.sync.dma_start(out=pkb_t[:], in_=pkb.ap())
        hts_t = consts.tile([V, 2 * NB * R], bf)
        nc.sync.dma_start(out=hts_t[:], in_=hts.ap())

        # ---- span gathers (exact cover, sorted spans) ----
        gA = [None] * NB   # wide: [128, 2, 4D] two quads
        gB = [None] * NB   # narrow: [128, 3, 2D] three pairs
        for h in range(NB):
            c0 = h * GCOLS
            if variant == "exact":
                ga = work.tile([128, 2, 4 * D], bf, name=f"ga_{h}", tag=f"ga_{h}")
                g = nc.gpsimd.dma_gather(
                    out_ap=ga[:], in_ap=sent_view(4),
                    idxs_ap=idx_t[:, c0 : c0 + 16],
                    num_idxs=256, num_idxs_reg=256,
                    elem_size=4 * D, elem_step=D, single_packet=False)
                gb = work.tile([128, 3, 2 * D], bf, name=f"gb_{h}", tag=f"gb_{h}")
                nc.gpsimd.dma_gather(
                    out_ap=gb[:], in_ap=sent_view(2),
                    idxs_ap=idx_t[:, c0 + 16 : c0 + 40],
                    num_idxs=384, num_idxs_reg=384,
                    elem_size=2 * D, elem_step=D, single_packet=False)
            else:
                ga = work.tile([128, 1, 8 * D], bf, name=f"ga_{h}", tag=f"ga_{h}")
                g = nc.gpsimd.dma_gather(
                    out_ap=ga[:], in_ap=sent_view(8),
                    idxs_ap=idx_t[:, c0 : c0 + 8],
                    num_idxs=128, num_idxs_reg=128,
                    elem_size=8 * D, elem_step=D, single_packet=False)
                gb = work.tile([128, 1, 8 * D], bf, name=f"gb_{h}", tag=f"gb_{h}")
                nc.gpsimd.dma_gather(
                    out_ap=gb[:], in_ap=sent_view(8),
                    idxs_ap=idx_t[:, c0 + 8 : c0 + 16],
                    num_idxs=128, num_idxs_reg=128,
                    elem_size=8 * D, elem_step=D, single_packet=False)
            gA[h], gB[h] = ga, gb

        # big weight pack gated on batch-0's first gather completing: a dummy
        # copy reading gA[0] writes into wpk_t, so the DMA (WAW) waits for it
        wpk_t = consts.tile([128, WK_N], bf)
        nc.vector.tensor_copy(wpk_t[:, 0:1], gA[0][:, 0, 0:1])
        nc.sync.dma_start(out=wpk_t[:], in_=wpk.ap())

        # ---- max trees ----
        def mask(h, cc, j):
            c = (h * 2 + cc) * 7 + j
            return pkf_t[:, c : c + 1]

        sem_b = []
        for h in range(NB):
            sh = work.tile([128, 2, D], bf, name=f"sem_{h}", tag=f"sem_{h}")
            if variant == "exact":
                # [128, 4, 2, D]: 4 row-pairs across the two quads
                p4 = gA[h][:].rearrange("p q (r s d) -> p (q r) s d", r=2, s=2)
                t4 = work.tile([128, 4, D], bf, name=f"t4_{h}", tag=f"t4_{h}")
                nc.vector.tensor_tensor(
                    out=t4[:], in0=p4[:, :, 0, :], in1=p4[:, :, 1, :], op=MAX)
                p2 = t4[:].rearrange("p (r s) d -> p r s d", s=2)
                t2 = work.tile([128, 2, D], bf, name=f"t2_{h}", tag=f"t2_{h}")
                nc.vector.tensor_tensor(
                    out=t2[:], in0=p2[:, :, 0, :], in1=p2[:, :, 1, :], op=MAX)
                nc.vector.tensor_tensor(
                    out=sh[:, 0, :], in0=t2[:, 0, :], in1=t2[:, 1, :], op=MAX)
                # [128, 3, 2, D]: pairs d0,d1,d2
                p3 = gB[h][:].rearrange("p q (s d) -> p q s d", s=2)
                z0 = work.tile([128, D], bf, name=f"z0_{h}", tag=f"z0_{h}")
                nc.vector.scalar_tensor_tensor(
                    out=z0[:], in0=p3[:, 0, 1, :], scalar=mask(h, 1, 0),
                    in1=p3[:, 0, 0, :], op0=ADD, op1=MAX)
                u2 = work.tile([128, 2, D], bf, name=f"u2_{h}", tag=f"u2_{h}")
                nc.vector.tensor_tensor(
                    out=u2[:], in0=p3[:, 1:3, 0, :], in1=p3[:, 1:3, 1, :], op=MAX)
                nc.vector.tensor_tensor(
                    out=u2[:, 0, :], in0=u2[:, 0, :], in1=u2[:, 1, :], op=MAX)
                nc.vector.tensor_tensor(
                    out=sh[:, 1, :], in0=z0[:], in1=u2[:, 0, :], op=MAX)
            else:
                for cc, g in ((0, gA[h]), (1, gB[h])):
                    rows = g[:].rearrange("p one (r d) -> p (one r) d", r=8)
                    t01 = work.tile([128, D], bf, name=f"w{cc}a_{h}", tag=f"w{cc}a_{h}")
                    t23 = work.tile([128, D], bf, name=f"w{cc}b_{h}", tag=f"w{cc}b_{h}")
                    t45 = work.tile([128, D], bf, name=f"w{cc}c_{h}", tag=f"w{cc}c_{h}")
                    t67 = work.tile([128, D], bf, name=f"w{cc}d_{h}", tag=f"w{cc}d_{h}")
                    # masks j: 0=m1,1=m3,2=m2',3=m4',4=m5,5=m6,6=m7
                    nc.vector.scalar_tensor_tensor(out=t01[:], in0=rows[:, 1, :],
                        scalar=mask(h, cc, 0), in1=rows[:, 0, :], op0=ADD, op1=MAX)
                    nc.vector.scalar_tensor_tensor(out=t23[:], in0=rows[:, 3, :],
                        scalar=mask(h, cc, 1), in1=rows[:, 2, :], op0=ADD, op1=MAX)
                    nc.vector.scalar_tensor_tensor(out=t45[:], in0=rows[:, 5, :],
                        scalar=mask(h, cc, 4), in1=rows[:, 4, :], op0=ADD, op1=MAX)
                    nc.vector.scalar_tensor_tensor(out=t67[:], in0=rows[:, 7, :],
                        scalar=mask(h, cc, 6), in1=rows[:, 6, :], op0=ADD, op1=MAX)
                    nc.vector.scalar_tensor_tensor(out=t01[:], in0=t23[:],
                        scalar=mask(h, cc, 2), in1=t01[:], op0=ADD, op1=MAX)
                    nc.vector.scalar_tensor_tensor(out=t45[:], in0=t67[:],
                        scalar=mask(h, cc, 5), in1=t45[:], op0=ADD, op1=MAX)
                    nc.vector.scalar_tensor_tensor(out=sh[:, cc, :], in0=t45[:],
                        scalar=mask(h, cc, 3), in1=t01[:], op0=ADD, op1=MAX)
            sem_b.append(sh)

        def w1c(m):
            return wpk_t[:, m * HID : (m + 1) * HID]

        def w2c(k):
            return wpk_t[:, WK_W2 + k * REL : WK_W2 + (k + 1) * REL]

        def hsel(b):
            return hts_t[:, b * R : (b + 1) * R]

        def tsel(b):
            return hts_t[:, (NB + b) * R : (NB + b + 1) * R]

        ident = pkb_t[:, PK_ID : PK_ID + 128]

        # ---- per-batch compute ----
        out_sb = work.tile([128, NB, R], bf)
        for b in range(NB):
            inv = pkf_t[:, FK_INV + b : FK_INV + b + 1]
            ps_v = psums.tile([128, D], mybir.dt.float32, space="PSUM",
                              tag="ps_v", bufs=1, name="ps_v")
            for cc in range(2):
                pt = pkb_t[:, PK_PT + (b * 2 + cc) * V : PK_PT + (b * 2 + cc + 1) * V]
                for n0, nsz in ((0, 512), (512, 256)):
                    nc.tensor.matmul(
                        ps_v[:, n0 : n0 + nsz], lhsT=pt,
                        rhs=sem_b[b][:, cc, n0 : n0 + nsz],
                        start=(cc == 0), stop=(cc == 1))
            v_sb = perb.tile([V, D], bf, tag="v_sb")
            nc.scalar.activation(v_sb[:], ps_v[:], AF.Copy, scale=inv)

            # V_emb.T via PE transposes, packed 4 and 2 to a PSUM bank
            vt_sb = perb.tile([128, 6, V], bf, tag="vt_sb")
            for g0, ntr in ((0, 4), (4, 2)):
                ps_tr = psums.tile([128, 1024], bf, space="PSUM",
                                   tag="tr", bufs=2, name="ps_tr")
                for k in range(ntr):
                    m = g0 + k
                    nc.tensor.transpose(ps_tr[:, k * 128 : (k + 1) * 128],
                                        v_sb[:, m * 128 : (m + 1) * 128], ident)
                nc.scalar.copy(vt_sb[:, g0 : g0 + ntr, :], ps_tr[:, : ntr * 128])

            # head/tail (one-hot matmuls) + product
            head_t = perb.tile([128, 6, R], bf, tag="head_t")
            tail_t = perb.tile([128, 6, R], bf, tag="tail_t")
            prod_t = perb.tile([128, 6, R], bf, tag="prod_t")
            for m in range(6):
                ps_h = psum_tile("ps_h", "sel", 2)
                nc.tensor.matmul(ps_h[:], lhsT=v_sb[:, m * 128 : (m + 1) * 128],
                                 rhs=hsel(b), start=True, stop=True)
                if m % 2 == 0:
                    nc.vector.tensor_copy(head_t[:, m, :], ps_h[:])
                else:
                    nc.scalar.copy(head_t[:, m, :], ps_h[:])
                ps_t2 = psum_tile("ps_t2", "sel", 2)
                nc.tensor.matmul(ps_t2[:], lhsT=v_sb[:, m * 128 : (m + 1) * 128],
                                 rhs=tsel(b), start=True, stop=True)
                if m % 2 == 0:
                    nc.scalar.copy(tail_t[:, m, :], ps_t2[:])
                else:
                    nc.vector.tensor_copy(tail_t[:, m, :], ps_t2[:])
                nc.vector.tensor_tensor(out=prod_t[:, m, :], in0=head_t[:, m, :],
                                        in1=tail_t[:, m, :], op=MUL)

            # Vw_a / Vw_c = V_emb @ W1a/c  (V_emb already 1/cnt-scaled)
            vw_a = perb.tile([V, HID], bf, tag="vw_a")
            vw_c = perb.tile([V, HID], bf, tag="vw_c")
            for vw, c0 in ((vw_a, 0), (vw_c, 6)):
                ps_vw = psum_tile("ps_vw", "sel", 2)
                for m in range(6):
                    nc.tensor.matmul(ps_vw[:, :HID], lhsT=vt_sb[:, m, :],
                                     rhs=w1c(c0 + m), start=(m == 0), stop=(m == 5))
                nc.scalar.activation(vw[:], ps_vw[:, :HID], AF.Copy)

            # hidden = relu(sum of five blocks), transposed [HID, R]
            hid_t = perb.tile([128, 3, R], bf, tag="hid_t")
            for m3 in range(3):
                msl = slice(m3 * 128, (m3 + 1) * 128)
                chunks = [(vw_a[:, msl], hsel(b)), (vw_c[:, msl], tsel(b)),
                          (pkb_t[:40, PK_EW + m3 * 128 : PK_EW + (m3 + 1) * 128],
                           pkb_t[:40, PK_ES + b * R : PK_ES + (b + 1) * R])]
                chunks += [(w1c(12 + m)[:, msl], prod_t[:, m, :]) for m in range(6)]
                ps_hid = psum_tile("ps_hid", "hid", 2)
                for i, (lhsT, rhs_ap) in enumerate(chunks):
                    nc.tensor.matmul(ps_hid[:], lhsT=lhsT, rhs=rhs_ap,
                                     start=(i == 0), stop=(i == len(chunks) - 1))
                nc.scalar.activation(hid_t[:, m3, :], ps_hid[:], AF.Relu)

            # out = W2.T @ hid + b2
            ps_o = psum_tile("ps_o", "hid", 2)
            for kc in range(W2C):
                nc.tensor.matmul(ps_o[:REL, :], lhsT=w2c(kc), rhs=hid_t[:, kc, :],
                                 start=(kc == 0), stop=(kc == W2C - 1))
            nc.scalar.activation(out_sb[:REL, b, :], ps_o[:REL, :], AF.Identity,
                                 bias=pkf_t[:REL, FK_B2 : FK_B2 + 1])
            nc.sync.dma_start(out=outd.ap()[:, b, :], in_=out_sb[:REL, b, :])

    nc.compile()
    return nc


def _idx_table(flat):
    """Gather index table: n descs -> [128, n//16] int16, 16-partition wrap."""
    n = len(flat)
    return np.tile(flat.astype(np.int16).reshape(n // 16, 16).T, (8, 1))


def _prep_core(c, sentence_repr, esi, vidx, vmask, ht, dis_h, dis_t,
               ew2, wpk_a, b2_f, force_variant=None):
    """Per-core input map for batches [c*NB, c*NB+NB). Returns (inputs, variant)."""
    bs = range(c * NB, c * NB + NB)

    sent = np.empty((SENT_ROWS, D), dtype=BF16)
    for j, b in enumerate(bs):
        sent[j * S : (j + 1) * S] = sentence_repr[b].astype(BF16)
    sent[NEGROW:] = BF16(NEG)

    starts = np.stack([esi[b, :, 0] for b in bs])
    widths = np.stack([esi[b, :, 1] - esi[b, :, 0] for b in bs])

    perms, variant = [], force_variant or "exact"
    for h in range(NB):
        p = np.argsort(-widths[h], kind="stable")
        perms.append(p)
        w_s = widths[h][p]
        if force_variant is None and (w_s[:128].min() < 3 or w_s[128:].max() > 4):
            variant = "masked88"

    gidx = np.zeros((128, NB * GCOLS), dtype=np.int16)
    masks = np.zeros((128, NB, 2, 7), dtype=np.float32)
    for h in range(NB):
        st_s = starts[h][perms[h]] + h * S
        w_s = widths[h][perms[h]]
        c0 = h * GCOLS
        if variant == "exact":
            stw, ww = st_s[:128], w_s[:128]          # wide: w>=3
            stn, wn = st_s[128:], w_s[128:]          # narrow: w<=4
            qa = np.concatenate([stw, stw + ww - 3])                 # 256 quads
            d0 = stn
            d1 = np.where(wn >= 1, stn + wn - 1, NEGROW)
            d2 = np.where(wn >= 3, stn + 1, np.where(wn >= 1, stn + wn - 1, NEGROW))
            pb = np.concatenate([d0, d1, d2])                        # 384 pairs
            gidx[:, c0 : c0 + 16] = _idx_table(qa)
            gidx[:, c0 + 16 : c0 + 40] = _idx_table(pb)
            masks[:, h, 1, 0] = np.where(wn < 1, np.float32(NEG), 0.0)
        else:
            gidx[:, c0 : c0 + 8] = _idx_table(st_s[:128])
            gidx[:, c0 + 8 : c0 + 16] = _idx_table(st_s[128:])
            MJ = [1, 3, 2, 4, 5, 6, 7]
            for cc in range(2):
                wq = w_s[cc * 128 : (cc + 1) * 128]
                for j, thr in enumerate(MJ):
                    masks[:, h, cc, j] = np.where(wq < thr, np.float32(NEG), 0.0)

    pkf = np.zeros((128, FK_N), dtype=np.float32)
    pkf[:, :FK_INV] = masks.reshape(128, -1)
    pkb = np.zeros((128, PK_N), dtype=BF16)
    pkb[:, PK_ID : PK_ID + 128] = np.eye(128, dtype=BF16)
    pkb[:40, PK_EW : PK_EW + HID] = ew2
    hts_a = np.zeros((V, 2 * NB * R), dtype=BF16)
    for j, b in enumerate(bs):
        pt = np.zeros((NS, V), dtype=np.float32)
        np.add.at(pt, (vidx[b].ravel(), np.repeat(np.arange(V), C)),
                  vmask[b].ravel().astype(np.float32))
        pt = pt[perms[j]]
        pkb[:, PK_PT + (j * 2) * V : PK_PT + (j * 2 + 2) * V] = (
            pt.reshape(2, 128, V).transpose(1, 0, 2).reshape(128, 2 * V).astype(BF16))
        pkf[:, FK_INV + j] = 1.0 / np.maximum(vmask[b].sum(axis=1).astype(np.float32), 1.0)
        hts_a[ht[b, :, 0], j * R + np.arange(R)] = BF16(1.0)
        hts_a[ht[b, :, 1], (NB + j) * R + np.arange(R)] = BF16(1.0)
        es = np.zeros((40, R), dtype=BF16)
        es[dis_h[b], np.arange(R)] = BF16(1.0)
        es[DIS + dis_t[b], np.arange(R)] = BF16(1.0)
        pkb[:40, PK_ES + j * R : PK_ES + (j + 1) * R] = es
    pkf[:REL, FK_B2] = b2_f

    return dict(sent=sent, gidx=gidx, pkf=pkf, pkb=pkb, hts=hts_a, wpk=wpk_a), variant


def run(trace=False, **inputs):
    sentence_repr = np.asarray(inputs["sentence_repr"], dtype=np.float32)
    esi = np.asarray(inputs["entity_span_indices"]).astype(np.int64)
    vidx = np.asarray(inputs["vertex_indices"]).astype(np.int64)
    vmask = np.asarray(inputs["vertex_indices_mask"]).astype(np.int64)
    ht = np.asarray(inputs["head_tail_indices"]).astype(np.int64)
    dis_h = np.asarray(inputs["dis_h_2_t"]).astype(np.int64)
    dis_t = np.asarray(inputs["dis_t_2_h"]).astype(np.int64)
    dis_embed = np.asarray(inputs["dis_embed"], dtype=np.float32)
    w1 = np.asarray(inputs["W1"], dtype=np.float32)
    w2 = np.asarray(inputs["W2"], dtype=np.float32)
    b2 = np.asarray(inputs["b2"], dtype=np.float32)

    fin = D + DIS
    ew2 = np.concatenate([dis_embed @ w1[D : D + DIS],
                          dis_embed @ w1[fin + D : fin + D + DIS]], axis=0).astype(BF16)
    w1_abc = np.concatenate([w1[0:D], w1[fin : fin + D], w1[2 * fin : 2 * fin + D]], axis=0)
    w1_p = np.ascontiguousarray(
        w1_abc.astype(BF16).reshape(NW1C, 128, HID).transpose(1, 0, 2)).reshape(128, -1)
    w2_p = np.ascontiguousarray(
        w2.astype(BF16).reshape(W2C, 128, REL).transpose(1, 0, 2)).reshape(128, -1)
    wpk_a = np.concatenate([w1_p, w2_p], axis=1)
    assert wpk_a.shape == (128, WK_N)

    in_maps, variant = [], "exact"
    for c in range(NCORES):
        m, vr = _prep_core(c, sentence_repr, esi, vidx, vmask, ht, dis_h, dis_t,
                           ew2, wpk_a, b2)
        in_maps.append(m)
        if vr != "exact":
            variant = vr
    if variant != "exact":
        # rebuild per-core tables consistently for the fallback variant
        in_maps = []
        for c in range(NCORES):
            m, _ = _prep_core(c, sentence_repr, esi, vidx, vmask, ht, dis_h, dis_t,
                              ew2, wpk_a, b2, force_variant=variant)
            in_maps.append(m)

    if variant not in _NC_CACHE:
        _NC_CACHE[variant] = _build(variant)

    res = bass_utils.run_bass_kernel_spmd(
        _NC_CACHE[variant], in_maps, core_ids=list(range(NCORES)), trace=trace
    )

    out = np.empty((B, R, REL), dtype=np.float32)
    for c in range(NCORES):
        o = np.asarray(res.results[c]["outd"], dtype=np.float32)  # (REL, NB, R)
        for j in range(NB):
            out[c * NB + j] = o[:, j].T
    return out, res


def kernel(**inputs):
    out, _ = run(**inputs)
    return out


# revision 25
# speedup vs baseline: 1.0981x; 1.0981x over previous
"""Trainium2 Bass kernel for nn_BiLSTM_M_61615600828569 (segment_reduce).

Full computation per batch:
  span_emb = masked-max-pool of token windows   (B,256,768)
  vertex_emb = masked-mean over coref spans     (B,128,768)
  head/tail  = vertex gather by relation        (B,512,768)
  feat = [head, eh, tail, et, head*tail]        (B,512,2344)
  out  = relu(feat @ W1) @ W2 + b2              (B,512,97)

Sharding: data-parallel over batch; 16 batches / 8 cores = 2 per core.
All index work is precomputed on host; float math runs on device in bf16
with fp32 PSUM accumulation, in transposed layout (features on
partitions) so the final predict.T has the 97 classes on partitions for
a per-partition bias add.

Span pooling: spans are sorted by width per batch (the permutation is
folded into the host-built pool matrix, so it is free).  The widest 128
spans ("cc0", width>=3 whp) are fetched as two exact-cover 4-row quads
(start and start+w-3); the narrowest 128 ("cc1", width<=4 whp) as three
exact-cover 2-row pairs.  Exact cover means the max tree needs no row
masks (duplicated rows are harmless under max), except one leaf mask
for width-0 spans; mask-free levels run as single strided DVE ops since
DVE cost is dominated by a ~1us per-instruction overhead.  A rare
width distribution that breaks the cc0/cc1 bounds (~4-sigma) falls back
to a masked 8-row variant.

DMA schedule: only small tables load up front so the span gathers own
the HBM bandwidth; the big W1/W2 load is released by a manual semaphore
bumped by batch-0's first gather completion.  dis_embed@W1 blocks are
folded on host into one 40-row contraction; V_emb.T comes from PE
transposes packed four-to-a-PSUM-bank.
"""
import numpy as np
import ml_dtypes
from contextlib import ExitStack

import concourse.bass as bass
import concourse.bacc as bacc
import concourse.tile as tile
from concourse import mybir
from concourse import bass_utils
from concourse import library_config

BF16 = ml_dtypes.bfloat16

B, S, D = 16, 1024, 768
NS, MAXW = 256, 8
V, C = 128, 6
R = 512
REL, HID, DIS = 97, 384, 20
NEG = -2e30

NCORES = 8
NB = B // NCORES          # batches per core = 2
NEGROW = NB * S           # two -inf rows appended for width-0 pair redirects
SENT_ROWS = NB * S + 2
NW1C = 18                 # W1 chunks: a(6) + c(6) + e(6)
W2C = HID // 128          # 3
GCOLS = 40                # gidx cols per batch: 16 (quads) + 24 (pairs)

# bf16 "small" pack layout (columns)
PK_ID = 0                 # identity [128,128]
PK_PT = 128               # poolt    [128, NB*2*V=512]
PK_EW = 640               # ew2      [40, 384]
PK_ES = 1024              # esel2    [40, NB*R=1024]
PK_N = 2048
# f32 pack layout (columns)
FK_MASK = 0               # masks [128, NB*2*7]
FK_INV = 28               # invcnt [128, NB]
FK_B2 = 30                # b2 [97, 1]
FK_N = 31
# w1/w2 pack (bf16) columns
WK_W1 = 0                 # [128, 18*384]
WK_W2 = NW1C * 384        # [128, 3*97]
WK_N = WK_W2 + W2C * REL


def _patch_drain_and_barrier():
    """Walrus rejects >1 explicit sync wait on a Drain (TPB_CTRL), but Tile's
    tail drain waits on every used proc sem at once. Emit one single-wait
    drain per proc instead; the final drain then needs no waits."""
    import concourse.tile as tile_mod
    from concourse.vector_clock import VectorClock, ScopedClock

    if getattr(tile_mod.TileContext, "_ant_drain_patched", False):
        return

    def _patched(self, tick_clock, wait_clock):
        full = tick_clock.global_clock
        n = len(full)
        engines = [self.nc.sync, self.nc.vector, self.nc.scalar,
                   self.nc.tensor, self.nc.gpsimd]
        for i, p in enumerate([q for q in range(n) if full[q] > 0]):
            vec = [full[q] if q == p else 0 for q in range(n)]
            d = engines[i % len(engines)].drain()
            wait_clock.add_sem_waits(d.ins, ScopedClock({None: VectorClock(vec)}))
        self.nc.sync.drain()
        self.nc.all_engine_barrier()
        popped = self.nc._tile_sem_poison_stack.pop()
        assert popped is self._sem_poison
        self.nc.clear_and_free_semaphores(list(self.sems.allocated().values()))
        self.nc.all_engine_barrier()

    tile_mod.TileContext._drain_and_barrier = _patched
    tile_mod.TileContext._ant_drain_patched = True


_patch_drain_and_barrier()

_NC_CACHE = {}


def _build(variant):
    """One-core program; SPMD-replicated across the 8 cores.
    variant: 'exact' (cc0 w>=3, cc1 w<=4) or 'masked88' fallback."""
    bf = mybir.dt.bfloat16
    f32 = mybir.dt.float32
    AF = mybir.ActivationFunctionType
    ADD = mybir.AluOpType.add
    MAX = mybir.AluOpType.max
    MUL = mybir.AluOpType.mult

    nc = bacc.Bacc("TRN2", target_bir_lowering=False, debug=False, num_devices=1)

    sent = nc.dram_tensor("sent", (SENT_ROWS, D), bf, kind="ExternalInput")
    gidx = nc.dram_tensor("gidx", (128, NB * GCOLS), mybir.dt.int16, kind="ExternalInput")
    pkf = nc.dram_tensor("pkf", (128, FK_N), f32, kind="ExternalInput")
    pkb = nc.dram_tensor("pkb", (128, PK_N), bf, kind="ExternalInput")
    hts = nc.dram_tensor("hts", (V, 2 * NB * R), bf, kind="ExternalInput")
    wpk = nc.dram_tensor("wpk", (128, WK_N), bf, kind="ExternalInput")
    outd = nc.dram_tensor("outd", (REL, NB, R), bf, kind="ExternalOutput")

    def sent_view(nrows):
        return bass.AP(tensor=sent.ap().tensor, offset=0,
                       ap=[[D, SENT_ROWS - (nrows - 1)], [1, nrows * D]])

    with tile.TileContext(nc) as tc, ExitStack() as ctx:
        consts = ctx.enter_context(tc.tile_pool(name="consts", bufs=1))
        work = ctx.enter_context(tc.tile_pool(name="work", bufs=1))
        perb = ctx.enter_context(tc.tile_pool(name="perb", bufs=2))
        psums = ctx.enter_context(tc.tile_pool(name="psums", bufs=1, space="PSUM"))

        def psum_tile(name, tag, bufs):
            return psums.tile([128, R], mybir.dt.float32, space="PSUM",
                              tag=tag, bufs=bufs, name=name)

        idx_t = consts.tile([128, NB * GCOLS], mybir.dt.int16)
        nc.sync.dma_start(out=idx_t[:], in_=gidx.ap())
        pkf_t = consts.tile([128, FK_N], f32)
        nc.sync.dma_start(out=pkf_t[:], in_=pkf.ap())
        pkb_t = consts.tile([128, PK_N], bf)
        nc.sync.dma_start(out=pkb_t[:], in_=pkb.ap())
        hts_t = consts.tile([V, 2 * NB * R], bf)
        nc.sync.dma_start(out=hts_t[:], in_=hts.ap())
        wpk_t = consts.tile([128, WK_N], bf)
        nc.sync.dma_start(out=wpk_t[:], in_=wpk.ap())

        # ---- span gathers (exact cover, sorted spans) ----
        gA = [None] * NB   # wide: [128, 2, 4D] two quads
        gB = [None] * NB   # narrow: [128, 3, 2D] three pairs
        for h in range(NB):
            c0 = h * GCOLS
            if variant == "exact":
                ga = work.tile([128, 2, 4 * D], bf, name=f"ga_{h}", tag=f"ga_{h}")
                g = nc.gpsimd.dma_gather(
                    out_ap=ga[:], in_ap=sent_view(4),
                    idxs_ap=idx_t[:, c0 : c0 + 16],
                    num_idxs=256, num_idxs_reg=256,
                    elem_size=4 * D, elem_step=D, single_packet=False)
                gb = work.tile([128, 3, 2 * D], bf, name=f"gb_{h}", tag=f"gb_{h}")
                nc.gpsimd.dma_gather(
                    out_ap=gb[:], in_ap=sent_view(2),
                    idxs_ap=idx_t[:, c0 + 16 : c0 + 40],
                    num_idxs=384, num_idxs_reg=384,
                    elem_size=2 * D, elem_step=D, single_packet=False)
            else:
                ga = work.tile([128, 1, 8 * D], bf, name=f"ga_{h}", tag=f"ga_{h}")
                g = nc.gpsimd.dma_gather(
                    out_ap=ga[:], in_ap=sent_view(8),
                    idxs_ap=idx_t[:, c0 : c0 + 8],
                    num_idxs=128, num_idxs_reg=128,
                    elem_size=8 * D, elem_step=D, single_packet=False)
                gb = work.tile([128, 1, 8 * D], bf, name=f"gb_{h}", tag=f"gb_{h}")
                nc.gpsimd.dma_gather(
                    out_ap=gb[:], in_ap=sent_view(8),
                    idxs_ap=idx_t[:, c0 + 8 : c0 + 16],
                    num_idxs=128, num_idxs_reg=128,
                    elem_size=8 * D, elem_step=D, single_packet=False)
            gA[h], gB[h] = ga, gb

        # ---- max trees ----
        def mask(h, cc, j):
            c = (h * 2 + cc) * 7 + j
            return pkf_t[:, c : c + 1]

        def tree(h):
            sh = work.tile([128, 2, D], bf, name=f"sem_{h}", tag=f"sem_{h}")
            if variant == "exact":
                # [128, 4, 2, D]: 4 row-pairs across the two quads
                p4 = gA[h][:].rearrange("p q (r s d) -> p (q r) s d", r=2, s=2)
                t4 = work.tile([128, 4, D], bf, name=f"t4_{h}", tag=f"t4_{h}")
                nc.vector.tensor_tensor(
                    out=t4[:], in0=p4[:, :, 0, :], in1=p4[:, :, 1, :], op=MAX)
                p2 = t4[:].rearrange("p (r s) d -> p r s d", s=2)
                t2 = work.tile([128, 2, D], bf, name=f"t2_{h}", tag=f"t2_{h}")
                nc.vector.tensor_tensor(
                    out=t2[:], in0=p2[:, :, 0, :], in1=p2[:, :, 1, :], op=MAX)
                nc.vector.tensor_tensor(
                    out=sh[:, 0, :], in0=t2[:, 0, :], in1=t2[:, 1, :], op=MAX)
                # [128, 3, 2, D]: pairs d0,d1,d2
                p3 = gB[h][:].rearrange("p q (s d) -> p q s d", s=2)
                z0 = work.tile([128, D], bf, name=f"z0_{h}", tag=f"z0_{h}")
                nc.vector.scalar_tensor_tensor(
                    out=z0[:], in0=p3[:, 0, 1, :], scalar=mask(h, 1, 0),
                    in1=p3[:, 0, 0, :], op0=ADD, op1=MAX)
                u2 = work.tile([128, 2, D], bf, name=f"u2_{h}", tag=f"u2_{h}")
                nc.vector.tensor_tensor(
                    out=u2[:], in0=p3[:, 1:3, 0, :], in1=p3[:, 1:3, 1, :], op=MAX)
                nc.vector.tensor_tensor(
                    out=u2[:, 0, :], in0=u2[:, 0, :], in1=u2[:, 1, :], op=MAX)
                nc.vector.tensor_tensor(
                    out=sh[:, 1, :], in0=z0[:], in1=u2[:, 0, :], op=MAX)
            else:
                for cc, g in ((0, gA[h]), (1, gB[h])):
                    rows = g[:].rearrange("p one (r d) -> p (one r) d", r=8)
                    t01 = work.tile([128, D], bf, name=f"w{cc}a_{h}", tag=f"w{cc}a_{h}")
                    t23 = work.tile([128, D], bf, name=f"w{cc}b_{h}", tag=f"w{cc}b_{h}")
                    t45 = work.tile([128, D], bf, name=f"w{cc}c_{h}", tag=f"w{cc}c_{h}")
                    t67 = work.tile([128, D], bf, name=f"w{cc}d_{h}", tag=f"w{cc}d_{h}")
                    # masks j: 0=m1,1=m3,2=m2',3=m4',4=m5,5=m6,6=m7
                    nc.vector.scalar_tensor_tensor(out=t01[:], in0=rows[:, 1, :],
                        scalar=mask(h, cc, 0), in1=rows[:, 0, :], op0=ADD, op1=MAX)
                    nc.vector.scalar_tensor_tensor(out=t23[:], in0=rows[:, 3, :],
                        scalar=mask(h, cc, 1), in1=rows[:, 2, :], op0=ADD, op1=MAX)
                    nc.vector.scalar_tensor_tensor(out=t45[:], in0=rows[:, 5, :],
                        scalar=mask(h, cc, 4), in1=rows[:, 4, :], op0=ADD, op1=MAX)
                    nc.vector.scalar_tensor_tensor(out=t67[:], in0=rows[:, 7, :],
                        scalar=mask(h, cc, 6), in1=rows[:, 6, :], op0=ADD, op1=MAX)
                    nc.vector.scalar_tensor_tensor(out=t01[:], in0=t23[:],
                        scalar=mask(h, cc, 2), in1=t01[:], op0=ADD, op1=MAX)
                    nc.vector.scalar_tensor_tensor(out=t45[:], in0=t67[:],
                        scalar=mask(h, cc, 5), in1=t45[:], op0=ADD, op1=MAX)
                    nc.vector.scalar_tensor_tensor(out=sh[:, cc, :], in0=t45[:],
                        scalar=mask(h, cc, 3), in1=t01[:], op0=ADD, op1=MAX)
            return sh

        def w1c(m):
            return wpk_t[:, m * HID : (m + 1) * HID]

        def w2c(k):
            return wpk_t[:, WK_W2 + k * REL : WK_W2 + (k + 1) * REL]

        def hsel(b):
            return hts_t[:, b * R : (b + 1) * R]

        def tsel(b):
            return hts_t[:, (NB + b) * R : (NB + b + 1) * R]

        ident = pkb_t[:, PK_ID : PK_ID + 128]

        # ---- per-batch tree + compute (interleaved per batch so batch-1's
        #      tree isn't queued behind batch-0's Vector copies) ----
        out_sb = work.tile([128, NB, R], bf)
        sem_b = {}
        for b in range(NB):
            sem_b[b] = tree(b)
            inv = pkf_t[:, FK_INV + b : FK_INV + b + 1]
            ps_v = psums.tile([128, D], mybir.dt.float32, space="PSUM",
                              tag="ps_v", bufs=1, name="ps_v")
            for cc in range(2):
                pt = pkb_t[:, PK_PT + (b * 2 + cc) * V : PK_PT + (b * 2 + cc + 1) * V]
                for n0, nsz in ((0, 512), (512, 256)):
                    nc.tensor.matmul(
                        ps_v[:, n0 : n0 + nsz], lhsT=pt,
                        rhs=sem_b[b][:, cc, n0 : n0 + nsz],
                        start=(cc == 0), stop=(cc == 1))
            v_sb = perb.tile([V, D], bf, tag="v_sb")
            nc.scalar.activation(v_sb[:], ps_v[:], AF.Copy, scale=inv)

            # V_emb.T via PE transposes, packed 4 and 2 to a PSUM bank
            vt_sb = perb.tile([128, 6, V], bf, tag="vt_sb")
            for g0, ntr in ((0, 4), (4, 2)):
                ps_tr = psums.tile([128, 1024], bf, space="PSUM",
                                   tag="tr", bufs=2, name="ps_tr")
                for k in range(ntr):
                    m = g0 + k
                    nc.tensor.transpose(ps_tr[:, k * 128 : (k + 1) * 128],
                                        v_sb[:, m * 128 : (m + 1) * 128], ident)
                nc.scalar.copy(vt_sb[:, g0 : g0 + ntr, :], ps_tr[:, : ntr * 128])

            # head/tail (one-hot matmuls) + product
            head_t = perb.tile([128, 6, R], bf, tag="head_t")
            tail_t = perb.tile([128, 6, R], bf, tag="tail_t")
            prod_t = perb.tile([128, 6, R], bf, tag="prod_t")
            for m in range(6):
                ps_h = psum_tile("ps_h", "sel", 2)
                nc.tensor.matmul(ps_h[:], lhsT=v_sb[:, m * 128 : (m + 1) * 128],
                                 rhs=hsel(b), start=True, stop=True)
                if m % 2 == 0:
                    nc.vector.tensor_copy(head_t[:, m, :], ps_h[:])
                else:
                    nc.scalar.copy(head_t[:, m, :], ps_h[:])
                ps_t2 = psum_tile("ps_t2", "sel", 2)
                nc.tensor.matmul(ps_t2[:], lhsT=v_sb[:, m * 128 : (m + 1) * 128],
                                 rhs=tsel(b), start=True, stop=True)
                if m % 2 == 0:
                    nc.scalar.copy(tail_t[:, m, :], ps_t2[:])
                else:
                    nc.vector.tensor_copy(tail_t[:, m, :], ps_t2[:])
                nc.vector.tensor_tensor(out=prod_t[:, m, :], in0=head_t[:, m, :],
                                        in1=tail_t[:, m, :], op=MUL)

            # Vw_a / Vw_c = V_emb @ W1a/c  (V_emb already 1/cnt-scaled)
            vw_a = perb.tile([V, HID], bf, tag="vw_a")
            vw_c = perb.tile([V, HID], bf, tag="vw_c")
            for vw, c0 in ((vw_a, 0), (vw_c, 6)):
                ps_vw = psum_tile("ps_vw", "sel", 2)
                for m in range(6):
                    nc.tensor.matmul(ps_vw[:, :HID], lhsT=vt_sb[:, m, :],
                                     rhs=w1c(c0 + m), start=(m == 0), stop=(m == 5))
                nc.scalar.activation(vw[:], ps_vw[:, :HID], AF.Copy)

            # hidden = relu(sum of five blocks), transposed [HID, R]
            hid_t = perb.tile([128, 3, R], bf, tag="hid_t")
            for m3 in range(3):
                msl = slice(m3 * 128, (m3 + 1) * 128)
                chunks = [(vw_a[:, msl], hsel(b)), (vw_c[:, msl], tsel(b)),
                          (pkb_t[:40, PK_EW + m3 * 128 : PK_EW + (m3 + 1) * 128],
                           pkb_t[:40, PK_ES + b * R : PK_ES + (b + 1) * R])]
                chunks += [(w1c(12 + m)[:, msl], prod_t[:, m, :]) for m in range(6)]
                ps_hid = psum_tile("ps_hid", "hid", 2)
                for i, (lhsT, rhs_ap) in enumerate(chunks):
                    nc.tensor.matmul(ps_hid[:], lhsT=lhsT, rhs=rhs_ap,
                                     start=(i == 0), stop=(i == len(chunks) - 1))
                nc.scalar.activation(hid_t[:, m3, :], ps_hid[:], AF.Relu)

            # out = W2.T @ hid + b2
            ps_o = psum_tile("ps_o", "hid", 2)
            for kc in range(W2C):
                nc.tensor.matmul(ps_o[:REL, :], lhsT=w2c(kc), rhs=hid_t[:, kc, :],
                                 start=(kc == 0), stop=(kc == W2C - 1))
            nc.scalar.activation(out_sb[:REL, b, :], ps_o[:REL, :], AF.Identity,
                                 bias=pkf_t[:REL, FK_B2 : FK_B2 + 1])
            nc.sync.dma_start(out=outd.ap()[:, b, :], in_=out_sb[:REL, b, :])

    nc.compile()
    return nc


def _idx_table(flat):
    """Gather index table: n descs -> [128, n//16] int16, 16-partition wrap."""
    n = len(flat)
    return np.tile(flat.astype(np.int16).reshape(n // 16, 16).T, (8, 1))


def _prep_core(c, sentence_repr, esi, vidx, vmask, ht, dis_h, dis_t,
               ew2, wpk_a, b2_f, force_variant=None):
    """Per-core input map for batches [c*NB, c*NB+NB). Returns (inputs, variant)."""
    bs = range(c * NB, c * NB + NB)

    sent = np.empty((SENT_ROWS, D), dtype=BF16)
    for j, b in enumerate(bs):
        sent[j * S : (j + 1) * S] = sentence_repr[b].astype(BF16)
    sent[NEGROW:] = BF16(NEG)

    starts = np.stack([esi[b, :, 0] for b in bs])
    widths = np.stack([esi[b, :, 1] - esi[b, :, 0] for b in bs])

    perms, variant = [], force_variant or "exact"
    for h in range(NB):
        p = np.argsort(-widths[h], kind="stable")
        perms.append(p)
        w_s = widths[h][p]
        if force_variant is None and (w_s[:128].min() < 3 or w_s[128:].max() > 4):
            variant = "masked88"

    gidx = np.zeros((128, NB * GCOLS), dtype=np.int16)
    masks = np.zeros((128, NB, 2, 7), dtype=np.float32)
    for h in range(NB):
        st_s = starts[h][perms[h]] + h * S
        w_s = widths[h][perms[h]]
        c0 = h * GCOLS
        if variant == "exact":
            stw, ww = st_s[:128], w_s[:128]          # wide: w>=3
            stn, wn = st_s[128:], w_s[128:]          # narrow: w<=4
            qa = np.concatenate([stw, stw + ww - 3])                 # 256 quads
            d0 = stn
            d1 = np.where(wn >= 1, stn + wn - 1, NEGROW)
            d2 = np.where(wn >= 3, stn + 1, np.where(wn >= 1, stn + wn - 1, NEGROW))
            pb = np.concatenate([d0, d1, d2])                        # 384 pairs
            gidx[:, c0 : c0 + 16] = _idx_table(qa)
            gidx[:, c0 + 16 : c0 + 40] = _idx_table(pb)
            masks[:, h, 1, 0] = np.where(wn < 1, np.float32(NEG), 0.0)
        else:
            gidx[:, c0 : c0 + 8] = _idx_table(st_s[:128])
            gidx[:, c0 + 8 : c0 + 16] = _idx_table(st_s[128:])
            MJ = [1, 3, 2, 4, 5, 6, 7]
            for cc in range(2):
                wq = w_s[cc * 128 : (cc + 1) * 128]
                for j, thr in enumerate(MJ):
                    masks[:, h, cc, j] = np.where(wq < thr, np.float32(NEG), 0.0)

    pkf = np.zeros((128, FK_N), dtype=np.float32)
    pkf[:, :FK_INV] = masks.reshape(128, -1)
    pkb = np.zeros((128, PK_N), dtype=BF16)
    pkb[:, PK_ID : PK_ID + 128] = np.eye(128, dtype=BF16)
    pkb[:40, PK_EW : PK_EW + HID] = ew2
    hts_a = np.zeros((V, 2 * NB * R), dtype=BF16)
    for j, b in enumerate(bs):
        pt = np.zeros((NS, V), dtype=np.float32)
        np.add.at(pt, (vidx[b].ravel(), np.repeat(np.arange(V), C)),
                  vmask[b].ravel().astype(np.float32))
        pt = pt[perms[j]]
        pkb[:, PK_PT + (j * 2) * V : PK_PT + (j * 2 + 2) * V] = (
            pt.reshape(2, 128, V).transpose(1, 0, 2).reshape(128, 2 * V).astype(BF16))
        pkf[:, FK_INV + j] = 1.0 / np.maximum(vmask[b].sum(axis=1).astype(np.float32), 1.0)
        hts_a[ht[b, :, 0], j * R + np.arange(R)] = BF16(1.0)
        hts_a[ht[b, :, 1], (NB + j) * R + np.arange(R)] = BF16(1.0)
        es = np.zeros((40, R), dtype=BF16)
        es[dis_h[b], np.arange(R)] = BF16(1.0)
        es[DIS + dis_t[b], np.arange(R)] = BF16(1.0)
        pkb[:40, PK_ES + j * R : PK_ES + (j + 1) * R] = es
    pkf[:REL, FK_B2] = b2_f

    return dict(sent=sent, gidx=gidx, pkf=pkf, pkb=pkb, hts=hts_a, wpk=wpk_a), variant


def run(trace=False, **inputs):
    sentence_repr = np.asarray(inputs["sentence_repr"], dtype=np.float32)
    esi = np.asarray(inputs["entity_span_indices"]).astype(np.int64)
    vidx = np.asarray(inputs["vertex_indices"]).astype(np.int64)
    vmask = np.asarray(inputs["vertex_indices_mask"]).astype(np.int64)
    ht = np.asarray(inputs["head_tail_indices"]).astype(np.int64)
    dis_h = np.asarray(inputs["dis_h_2_t"]).astype(np.int64)
    dis_t = np.asarray(inputs["dis_t_2_h"]).astype(np.int64)
    dis_embed = np.asarray(inputs["dis_embed"], dtype=np.float32)
    w1 = np.asarray(inputs["W1"], dtype=np.float32)
    w2 = np.asarray(inputs["W2"], dtype=np.float32)
    b2 = np.asarray(inputs["b2"], dtype=np.float32)

    fin = D + DIS
    ew2 = np.concatenate([dis_embed @ w1[D : D + DIS],
                          dis_embed @ w1[fin + D : fin + D + DIS]], axis=0).astype(BF16)
    w1_abc = np.concatenate([w1[0:D], w1[fin : fin + D], w1[2 * fin : 2 * fin + D]], axis=0)
    w1_p = np.ascontiguousarray(
        w1_abc.astype(BF16).reshape(NW1C, 128, HID).transpose(1, 0, 2)).reshape(128, -1)
    w2_p = np.ascontiguousarray(
        w2.astype(BF16).reshape(W2C, 128, REL).transpose(1, 0, 2)).reshape(128, -1)
    wpk_a = np.concatenate([w1_p, w2_p], axis=1)
    assert wpk_a.shape == (128, WK_N)

    in_maps, variant = [], "exact"
    for c in range(NCORES):
        m, vr = _prep_core(c, sentence_repr, esi, vidx, vmask, ht, dis_h, dis_t,
                           ew2, wpk_a, b2)
        in_maps.append(m)
        if vr != "exact":
            variant = vr
    if variant != "exact":
        # rebuild per-core tables consistently for the fallback variant
        in_maps = []
        for c in range(NCORES):
            m, _ = _prep_core(c, sentence_repr, esi, vidx, vmask, ht, dis_h, dis_t,
                              ew2, wpk_a, b2, force_variant=variant)
            in_maps.append(m)

    if variant not in _NC_CACHE:
        _NC_CACHE[variant] = _build(variant)

    res = bass_utils.run_bass_kernel_spmd(
        _NC_CACHE[variant], in_maps, core_ids=list(range(NCORES)), trace=trace
    )

    out = np.empty((B, R, REL), dtype=np.float32)
    for c in range(NCORES):
        o = np.asarray(res.results[c]["outd"], dtype=np.float32)  # (REL, NB, R)
        for j in range(NB):
            out[c * NB + j] = o[:, j].T
    return out, res


def kernel(**inputs):
    out, _ = run(**inputs)
    return out


# revision 33
# speedup vs baseline: 1.1557x; 1.0525x over previous
"""Trainium2 Bass kernel for nn_BiLSTM_M_61615600828569 (segment_reduce).

Full computation per batch:
  span_emb = masked-max-pool of token windows   (B,256,768)
  vertex_emb = masked-mean over coref spans     (B,128,768)
  head/tail  = vertex gather by relation        (B,512,768)
  feat = [head, eh, tail, et, head*tail]        (B,512,2344)
  out  = relu(feat @ W1) @ W2 + b2              (B,512,97)

Sharding: data-parallel over batch; 16 batches / 8 cores = 2 per core.
All index work is precomputed on host; float math runs on device in bf16
with fp32 PSUM accumulation, in transposed layout (features on
partitions) so the final predict.T has the 97 classes on partitions for
a per-partition bias add.

Span pooling: spans are sorted by width per batch (the permutation is
folded into the host-built pool matrix, so it is free).  The widest 128
spans ("cc0", width>=3 whp) are fetched as two exact-cover 4-row quads
(start and start+w-3); the narrowest 128 ("cc1", width<=4 whp) as three
exact-cover 2-row pairs.  Exact cover means the max tree needs no row
masks (duplicated rows are harmless under max), except one leaf mask
for width-0 spans; mask-free levels run as single strided DVE ops since
DVE cost is dominated by a ~1us per-instruction overhead.  A rare
width distribution that breaks the cc0/cc1 bounds (~4-sigma) falls back
to a masked 8-row variant.

DMA schedule: only small tables load up front so the span gathers own
the HBM bandwidth; the big W1/W2 load is released by a manual semaphore
bumped by batch-0's first gather completion.  dis_embed@W1 blocks are
folded on host into one 40-row contraction; V_emb.T comes from PE
transposes packed four-to-a-PSUM-bank.
"""
import numpy as np
import ml_dtypes
from contextlib import ExitStack

import concourse.bass as bass
import concourse.bacc as bacc
import concourse.tile as tile
from concourse import mybir
from concourse import bass_utils
from concourse import library_config

BF16 = ml_dtypes.bfloat16

B, S, D = 16, 1024, 768
NS, MAXW = 256, 8
V, C = 128, 6
R = 512
REL, HID, DIS = 97, 384, 20
NEG = -2e30

NCORES = 8
NB = B // NCORES          # batches per core = 2
NEGROW = NB * S           # two -inf rows appended for width-0 pair redirects
SENT_ROWS = NB * S + 2
NW1C = 18                 # W1 chunks: a(6) + c(6) + e(6)
W2C = HID // 128          # 3
GCOLS = 40                # gidx cols per batch: 16 (quads) + 24 (pairs)

# bf16 "small" pack layout (columns)
PK_ID = 0                 # identity [128,128]
PK_PT = 128               # poolt    [128, NB*2*V=512]
PK_EW = 640               # ew2      [40, 384]
PK_ES = 1024              # esel2    [40, NB*R=1024]
PK_N = 2048
# f32 pack layout (columns)
FK_MASK = 0               # masks [128, NB*2*7]
FK_INV = 28               # invcnt [128, NB]
FK_B2 = 30                # b2 [97, 1]
FK_N = 31
# w1/w2 pack (bf16) columns
WK_W1 = 0                 # [128, 18*384]
WK_W2 = NW1C * 384        # [128, 3*97]
WK_N = WK_W2 + W2C * REL


def _patch_drain_and_barrier():
    """Walrus rejects >1 explicit sync wait on a Drain (TPB_CTRL), but Tile's
    tail drain waits on every used proc sem at once. Emit one single-wait
    drain per proc instead; the final drain then needs no waits."""
    import concourse.tile as tile_mod
    from concourse.vector_clock import VectorClock, ScopedClock

    if getattr(tile_mod.TileContext, "_ant_drain_patched", False):
        return

    def _patched(self, tick_clock, wait_clock):
        full = tick_clock.global_clock
        n = len(full)
        engines = [self.nc.sync, self.nc.vector, self.nc.scalar,
                   self.nc.tensor, self.nc.gpsimd]
        for i, p in enumerate([q for q in range(n) if full[q] > 0]):
            vec = [full[q] if q == p else 0 for q in range(n)]
            d = engines[i % len(engines)].drain()
            wait_clock.add_sem_waits(d.ins, ScopedClock({None: VectorClock(vec)}))
        self.nc.sync.drain()
        self.nc.all_engine_barrier()
        popped = self.nc._tile_sem_poison_stack.pop()
        assert popped is self._sem_poison
        self.nc.clear_and_free_semaphores(list(self.sems.allocated().values()))
        self.nc.all_engine_barrier()

    tile_mod.TileContext._drain_and_barrier = _patched
    tile_mod.TileContext._ant_drain_patched = True


_patch_drain_and_barrier()

_NC_CACHE = {}


def _build(variant):
    """One-core program; SPMD-replicated across the 8 cores.
    variant: 'exact' (cc0 w>=3, cc1 w<=4) or 'masked88' fallback."""
    bf = mybir.dt.bfloat16
    f32 = mybir.dt.float32
    AF = mybir.ActivationFunctionType
    ADD = mybir.AluOpType.add
    MAX = mybir.AluOpType.max
    MUL = mybir.AluOpType.mult

    nc = bacc.Bacc("TRN2", target_bir_lowering=False, debug=False, num_devices=1)

    sent = nc.dram_tensor("sent", (SENT_ROWS, D), bf, kind="ExternalInput")
    gidx = nc.dram_tensor("gidx", (128, NB * GCOLS), mybir.dt.int16, kind="ExternalInput")
    pkf = nc.dram_tensor("pkf", (128, FK_N), f32, kind="ExternalInput")
    pkb = nc.dram_tensor("pkb", (128, PK_N), bf, kind="ExternalInput")
    hts = nc.dram_tensor("hts", (V, 2 * NB * R), bf, kind="ExternalInput")
    wpk = nc.dram_tensor("wpk", (128, WK_N), bf, kind="ExternalInput")
    outd = nc.dram_tensor("outd", (NB, REL, R), bf, kind="ExternalOutput")

    def sent_view(nrows):
        return bass.AP(tensor=sent.ap().tensor, offset=0,
                       ap=[[D, SENT_ROWS - (nrows - 1)], [1, nrows * D]])

    with tile.TileContext(nc) as tc, ExitStack() as ctx:
        consts = ctx.enter_context(tc.tile_pool(name="consts", bufs=1))
        work = ctx.enter_context(tc.tile_pool(name="work", bufs=1))
        perb = ctx.enter_context(tc.tile_pool(name="perb", bufs=2))
        psums = ctx.enter_context(tc.tile_pool(name="psums", bufs=1, space="PSUM"))

        def psum_tile(name, tag, bufs):
            return psums.tile([128, R], mybir.dt.float32, space="PSUM",
                              tag=tag, bufs=bufs, name=name)

        idx_t = consts.tile([128, NB * GCOLS], mybir.dt.int16)
        nc.sync.dma_start(out=idx_t[:], in_=gidx.ap())
        pkf_t = consts.tile([128, FK_N], f32)
        nc.sync.dma_start(out=pkf_t[:], in_=pkf.ap())
        pkb_t = consts.tile([128, PK_N], bf)
        nc.sync.dma_start(out=pkb_t[:], in_=pkb.ap())
        hts_t = consts.tile([V, 2 * NB * R], bf)
        nc.sync.dma_start(out=hts_t[:], in_=hts.ap())
        wpk_t = consts.tile([128, WK_N], bf)
        nc.sync.dma_start(out=wpk_t[:], in_=wpk.ap())

        # ---- span gathers (exact cover, sorted spans) ----
        gA = [None] * NB   # wide: two 4-row quad gathers (start / start+w-3)
        gB = [None] * NB   # narrow: [128, 3, 2D] three pairs
        for h in range(NB):
            c0 = h * GCOLS
            if variant == "exact":
                gs = work.tile([128, 1, 4 * D], bf, name=f"gs_{h}", tag=f"gs_{h}")
                nc.gpsimd.dma_gather(
                    out_ap=gs[:], in_ap=sent_view(4),
                    idxs_ap=idx_t[:, c0 : c0 + 8],
                    num_idxs=128, num_idxs_reg=128,
                    elem_size=4 * D, elem_step=D, single_packet=False)
                ge = work.tile([128, 1, 4 * D], bf, name=f"ge_{h}", tag=f"ge_{h}")
                nc.gpsimd.dma_gather(
                    out_ap=ge[:], in_ap=sent_view(4),
                    idxs_ap=idx_t[:, c0 + 8 : c0 + 16],
                    num_idxs=128, num_idxs_reg=128,
                    elem_size=4 * D, elem_step=D, single_packet=False)
                gb = work.tile([128, 3, 2 * D], bf, name=f"gb_{h}", tag=f"gb_{h}")
                nc.gpsimd.dma_gather(
                    out_ap=gb[:], in_ap=sent_view(2),
                    idxs_ap=idx_t[:, c0 + 16 : c0 + 40],
                    num_idxs=384, num_idxs_reg=384,
                    elem_size=2 * D, elem_step=D, single_packet=False)
                ga = (gs, ge)
            else:
                ga = work.tile([128, 1, 8 * D], bf, name=f"ga_{h}", tag=f"ga_{h}")
                nc.gpsimd.dma_gather(
                    out_ap=ga[:], in_ap=sent_view(8),
                    idxs_ap=idx_t[:, c0 : c0 + 8],
                    num_idxs=128, num_idxs_reg=128,
                    elem_size=8 * D, elem_step=D, single_packet=False)
                gb = work.tile([128, 1, 8 * D], bf, name=f"gb_{h}", tag=f"gb_{h}")
                nc.gpsimd.dma_gather(
                    out_ap=gb[:], in_ap=sent_view(8),
                    idxs_ap=idx_t[:, c0 + 8 : c0 + 16],
                    num_idxs=128, num_idxs_reg=128,
                    elem_size=8 * D, elem_step=D, single_packet=False)
            gA[h], gB[h] = ga, gb

        # ---- max trees ----
        def mask(h, cc, j):
            c = (h * 2 + cc) * 7 + j
            return pkf_t[:, c : c + 1]

        def tree(h):
            sh = work.tile([128, 2, D], bf, name=f"sem_{h}", tag=f"sem_{h}")
            if variant == "exact":
                # per quad-gather: [128, 2, 2, D] row-pairs, reduce to one row
                gs, ge = gA[h]
                halves = []
                for nm, g in (("s", gs), ("e", ge)):
                    pq = g[:].rearrange("p one (r s d) -> p (one r) s d", r=2, s=2)
                    tt = work.tile([128, 2, D], bf, name=f"t{nm}_{h}", tag=f"t{nm}_{h}")
                    nc.vector.tensor_tensor(
                        out=tt[:], in0=pq[:, :, 0, :], in1=pq[:, :, 1, :], op=MAX)
                    nc.vector.tensor_tensor(
                        out=tt[:, 0, :], in0=tt[:, 0, :], in1=tt[:, 1, :], op=MAX)
                    halves.append(tt)
                nc.vector.tensor_tensor(
                    out=sh[:, 0, :], in0=halves[0][:, 0, :], in1=halves[1][:, 0, :], op=MAX)
                # [128, 3, 2, D]: pairs d0,d1,d2
                p3 = gB[h][:].rearrange("p q (s d) -> p q s d", s=2)
                z0 = work.tile([128, D], bf, name=f"z0_{h}", tag=f"z0_{h}")
                nc.vector.scalar_tensor_tensor(
                    out=z0[:], in0=p3[:, 0, 1, :], scalar=mask(h, 1, 0),
                    in1=p3[:, 0, 0, :], op0=ADD, op1=MAX)
                u2 = work.tile([128, 2, D], bf, name=f"u2_{h}", tag=f"u2_{h}")
                nc.vector.tensor_tensor(
                    out=u2[:], in0=p3[:, 1:3, 0, :], in1=p3[:, 1:3, 1, :], op=MAX)
                nc.vector.tensor_tensor(
                    out=u2[:, 0, :], in0=u2[:, 0, :], in1=u2[:, 1, :], op=MAX)
                nc.vector.tensor_tensor(
                    out=sh[:, 1, :], in0=z0[:], in1=u2[:, 0, :], op=MAX)
            else:
                for cc, g in ((0, gA[h]), (1, gB[h])):
                    rows = g[:].rearrange("p one (r d) -> p (one r) d", r=8)
                    t01 = work.tile([128, D], bf, name=f"w{cc}a_{h}", tag=f"w{cc}a_{h}")
                    t23 = work.tile([128, D], bf, name=f"w{cc}b_{h}", tag=f"w{cc}b_{h}")
                    t45 = work.tile([128, D], bf, name=f"w{cc}c_{h}", tag=f"w{cc}c_{h}")
                    t67 = work.tile([128, D], bf, name=f"w{cc}d_{h}", tag=f"w{cc}d_{h}")
                    # masks j: 0=m1,1=m3,2=m2',3=m4',4=m5,5=m6,6=m7
                    nc.vector.scalar_tensor_tensor(out=t01[:], in0=rows[:, 1, :],
                        scalar=mask(h, cc, 0), in1=rows[:, 0, :], op0=ADD, op1=MAX)
                    nc.vector.scalar_tensor_tensor(out=t23[:], in0=rows[:, 3, :],
                        scalar=mask(h, cc, 1), in1=rows[:, 2, :], op0=ADD, op1=MAX)
                    nc.vector.scalar_tensor_tensor(out=t45[:], in0=rows[:, 5, :],
                        scalar=mask(h, cc, 4), in1=rows[:, 4, :], op0=ADD, op1=MAX)
                    nc.vector.scalar_tensor_tensor(out=t67[:], in0=rows[:, 7, :],
                        scalar=mask(h, cc, 6), in1=rows[:, 6, :], op0=ADD, op1=MAX)
                    nc.vector.scalar_tensor_tensor(out=t01[:], in0=t23[:],
                        scalar=mask(h, cc, 2), in1=t01[:], op0=ADD, op1=MAX)
                    nc.vector.scalar_tensor_tensor(out=t45[:], in0=t67[:],
                        scalar=mask(h, cc, 5), in1=t45[:], op0=ADD, op1=MAX)
                    nc.vector.scalar_tensor_tensor(out=sh[:, cc, :], in0=t45[:],
                        scalar=mask(h, cc, 3), in1=t01[:], op0=ADD, op1=MAX)
            return sh

        def w1c(m):
            return wpk_t[:, m * HID : (m + 1) * HID]

        def w2c(k):
            return wpk_t[:, WK_W2 + k * REL : WK_W2 + (k + 1) * REL]

        def hsel(b):
            return hts_t[:, b * R : (b + 1) * R]

        def tsel(b):
            return hts_t[:, (NB + b) * R : (NB + b + 1) * R]

        ident = pkb_t[:, PK_ID : PK_ID + 128]

        # ---- per-batch tree + compute (interleaved per batch so batch-1's
        #      tree isn't queued behind batch-0's Vector copies) ----
        out_sb = work.tile([128, NB, R], bf)
        sem_b = {}
        for b in range(NB):
            sem_b[b] = tree(b)
            inv = pkf_t[:, FK_INV + b : FK_INV + b + 1]
            ps_v = psums.tile([128, D], mybir.dt.float32, space="PSUM",
                              tag="ps_v", bufs=1, name="ps_v")
            for cc in range(2):
                pt = pkb_t[:, PK_PT + (b * 2 + cc) * V : PK_PT + (b * 2 + cc + 1) * V]
                for n0, nsz in ((0, 512), (512, 256)):
                    nc.tensor.matmul(
                        ps_v[:, n0 : n0 + nsz], lhsT=pt,
                        rhs=sem_b[b][:, cc, n0 : n0 + nsz],
                        start=(cc == 0), stop=(cc == 1))
            v_sb = perb.tile([V, D], bf, tag="v_sb")
            nc.scalar.activation(v_sb[:], ps_v[:], AF.Copy, scale=inv)

            # V_emb.T via PE transposes, packed 4 and 2 to a PSUM bank
            vt_sb = perb.tile([128, 6, V], bf, tag="vt_sb")
            for g0, ntr in ((0, 4), (4, 2)):
                ps_tr = psums.tile([128, 1024], bf, space="PSUM",
                                   tag="tr", bufs=1, name="ps_tr")
                for k in range(ntr):
                    m = g0 + k
                    nc.tensor.transpose(ps_tr[:, k * 128 : (k + 1) * 128],
                                        v_sb[:, m * 128 : (m + 1) * 128], ident)
                nc.scalar.copy(vt_sb[:, g0 : g0 + ntr, :], ps_tr[:, : ntr * 128])

            # head/tail (one-hot matmuls) + product
            head_t = perb.tile([128, 6, R], bf, tag="head_t")
            tail_t = perb.tile([128, 6, R], bf, tag="tail_t")
            prod_t = perb.tile([128, 6, R], bf, tag="prod_t")
            for m in range(6):
                ps_h = psum_tile("ps_h", "sel", 3)
                nc.tensor.matmul(ps_h[:], lhsT=v_sb[:, m * 128 : (m + 1) * 128],
                                 rhs=hsel(b), start=True, stop=True)
                if m % 2 == 0:
                    nc.vector.tensor_copy(head_t[:, m, :], ps_h[:])
                else:
                    nc.scalar.copy(head_t[:, m, :], ps_h[:])
                ps_t2 = psum_tile("ps_t2", "sel", 3)
                nc.tensor.matmul(ps_t2[:], lhsT=v_sb[:, m * 128 : (m + 1) * 128],
                                 rhs=tsel(b), start=True, stop=True)
                if m % 2 == 0:
                    nc.scalar.copy(tail_t[:, m, :], ps_t2[:])
                else:
                    nc.vector.tensor_copy(tail_t[:, m, :], ps_t2[:])
                nc.vector.tensor_tensor(out=prod_t[:, m, :], in0=head_t[:, m, :],
                                        in1=tail_t[:, m, :], op=MUL)

            # Vw_a / Vw_c = V_emb @ W1a/c  (V_emb already 1/cnt-scaled)
            vw_a = perb.tile([V, HID], bf, tag="vw_a")
            vw_c = perb.tile([V, HID], bf, tag="vw_c")
            for vw, c0 in ((vw_a, 0), (vw_c, 6)):
                ps_vw = psum_tile("ps_vw", "sel", 3)
                for m in range(6):
                    nc.tensor.matmul(ps_vw[:, :HID], lhsT=vt_sb[:, m, :],
                                     rhs=w1c(c0 + m), start=(m == 0), stop=(m == 5))
                nc.scalar.activation(vw[:], ps_vw[:, :HID], AF.Copy)

            # hidden = relu(sum of five blocks), transposed [HID, R]
            hid_t = perb.tile([128, 3, R], bf, tag="hid_t")
            for m3 in range(3):
                msl = slice(m3 * 128, (m3 + 1) * 128)
                chunks = [(vw_a[:, msl], hsel(b)), (vw_c[:, msl], tsel(b)),
                          (pkb_t[:40, PK_EW + m3 * 128 : PK_EW + (m3 + 1) * 128],
                           pkb_t[:40, PK_ES + b * R : PK_ES + (b + 1) * R])]
                chunks += [(w1c(12 + m)[:, msl], prod_t[:, m, :]) for m in range(6)]
                ps_hid = psum_tile("ps_hid", "hid", 2)
                for i, (lhsT, rhs_ap) in enumerate(chunks):
                    nc.tensor.matmul(ps_hid[:], lhsT=lhsT, rhs=rhs_ap,
                                     start=(i == 0), stop=(i == len(chunks) - 1))
                nc.scalar.activation(hid_t[:, m3, :], ps_hid[:], AF.Relu)

            # out = W2.T @ hid + b2
            ps_o = psum_tile("ps_o", "hid", 2)
            for kc in range(W2C):
                nc.tensor.matmul(ps_o[:REL, :], lhsT=w2c(kc), rhs=hid_t[:, kc, :],
                                 start=(kc == 0), stop=(kc == W2C - 1))
            nc.scalar.activation(out_sb[:REL, b, :], ps_o[:REL, :], AF.Identity,
                                 bias=pkf_t[:REL, FK_B2 : FK_B2 + 1])
            nc.sync.dma_start(out=outd.ap()[b], in_=out_sb[:REL, b, :])

    nc.compile()
    return nc


def _idx_table(flat):
    """Gather index table: n descs -> [128, n//16] int16, 16-partition wrap."""
    n = len(flat)
    return np.tile(flat.astype(np.int16).reshape(n // 16, 16).T, (8, 1))


def _prep_core(c, sentence_repr, esi, vidx, vmask, ht, dis_h, dis_t,
               ew2, wpk_a, b2_f, force_variant=None):
    """Per-core input map for batches [c*NB, c*NB+NB). Returns (inputs, variant)."""
    bs = range(c * NB, c * NB + NB)

    sent = np.empty((SENT_ROWS, D), dtype=BF16)
    for j, b in enumerate(bs):
        sent[j * S : (j + 1) * S] = sentence_repr[b].astype(BF16)
    sent[NEGROW:] = BF16(NEG)

    starts = np.stack([esi[b, :, 0] for b in bs])
    widths = np.stack([esi[b, :, 1] - esi[b, :, 0] for b in bs])

    perms, variant = [], force_variant or "exact"
    for h in range(NB):
        p = np.argsort(-widths[h], kind="stable")
        perms.append(p)
        w_s = widths[h][p]
        if force_variant is None and (w_s[:128].min() < 3 or w_s[128:].max() > 4):
            variant = "masked88"

    gidx = np.zeros((128, NB * GCOLS), dtype=np.int16)
    masks = np.zeros((128, NB, 2, 7), dtype=np.float32)
    for h in range(NB):
        st_s = starts[h][perms[h]] + h * S
        w_s = widths[h][perms[h]]
        c0 = h * GCOLS
        if variant == "exact":
            stw, ww = st_s[:128], w_s[:128]          # wide: w>=3
            stn, wn = st_s[128:], w_s[128:]          # narrow: w<=4
            d0 = stn
            d1 = np.where(wn >= 1, stn + wn - 1, NEGROW)
            d2 = np.where(wn >= 3, stn + 1, np.where(wn >= 1, stn + wn - 1, NEGROW))
            pb = np.concatenate([d0, d1, d2])                        # 384 pairs
            gidx[:, c0 : c0 + 8] = _idx_table(stw)
            gidx[:, c0 + 8 : c0 + 16] = _idx_table(stw + ww - 3)
            gidx[:, c0 + 16 : c0 + 40] = _idx_table(pb)
            masks[:, h, 1, 0] = np.where(wn < 1, np.float32(NEG), 0.0)
        else:
            gidx[:, c0 : c0 + 8] = _idx_table(st_s[:128])
            gidx[:, c0 + 8 : c0 + 16] = _idx_table(st_s[128:])
            MJ = [1, 3, 2, 4, 5, 6, 7]
            for cc in range(2):
                wq = w_s[cc * 128 : (cc + 1) * 128]
                for j, thr in enumerate(MJ):
                    masks[:, h, cc, j] = np.where(wq < thr, np.float32(NEG), 0.0)

    pkf = np.zeros((128, FK_N), dtype=np.float32)
    pkf[:, :FK_INV] = masks.reshape(128, -1)
    pkb = np.zeros((128, PK_N), dtype=BF16)
    pkb[:, PK_ID : PK_ID + 128] = np.eye(128, dtype=BF16)
    pkb[:40, PK_EW : PK_EW + HID] = ew2
    hts_a = np.zeros((V, 2 * NB * R), dtype=BF16)
    for j, b in enumerate(bs):
        pt = np.zeros((NS, V), dtype=np.float32)
        np.add.at(pt, (vidx[b].ravel(), np.repeat(np.arange(V), C)),
                  vmask[b].ravel().astype(np.float32))
        pt = pt[perms[j]]
        pkb[:, PK_PT + (j * 2) * V : PK_PT + (j * 2 + 2) * V] = (
            pt.reshape(2, 128, V).transpose(1, 0, 2).reshape(128, 2 * V).astype(BF16))
        pkf[:, FK_INV + j] = 1.0 / np.maximum(vmask[b].sum(axis=1).astype(np.float32), 1.0)
        hts_a[ht[b, :, 0], j * R + np.arange(R)] = BF16(1.0)
        hts_a[ht[b, :, 1], (NB + j) * R + np.arange(R)] = BF16(1.0)
        es = np.zeros((40, R), dtype=BF16)
        es[dis_h[b], np.arange(R)] = BF16(1.0)
        es[DIS + dis_t[b], np.arange(R)] = BF16(1.0)
        pkb[:40, PK_ES + j * R : PK_ES + (j + 1) * R] = es
    pkf[:REL, FK_B2] = b2_f

    return dict(sent=sent, gidx=gidx, pkf=pkf, pkb=pkb, hts=hts_a, wpk=wpk_a), variant


def run(trace=False, **inputs):
    sentence_repr = np.asarray(inputs["sentence_repr"], dtype=np.float32)
    esi = np.asarray(inputs["entity_span_indices"]).astype(np.int64)
    vidx = np.asarray(inputs["vertex_indices"]).astype(np.int64)
    vmask = np.asarray(inputs["vertex_indices_mask"]).astype(np.int64)
    ht = np.asarray(inputs["head_tail_indices"]).astype(np.int64)
    dis_h = np.asarray(inputs["dis_h_2_t"]).astype(np.int64)
    dis_t = np.asarray(inputs["dis_t_2_h"]).astype(np.int64)
    dis_embed = np.asarray(inputs["dis_embed"], dtype=np.float32)
    w1 = np.asarray(inputs["W1"], dtype=np.float32)
    w2 = np.asarray(inputs["W2"], dtype=np.float32)
    b2 = np.asarray(inputs["b2"], dtype=np.float32)

    fin = D + DIS
    ew2 = np.concatenate([dis_embed @ w1[D : D + DIS],
                          dis_embed @ w1[fin + D : fin + D + DIS]], axis=0).astype(BF16)
    w1_abc = np.concatenate([w1[0:D], w1[fin : fin + D], w1[2 * fin : 2 * fin + D]], axis=0)
    w1_p = np.ascontiguousarray(
        w1_abc.astype(BF16).reshape(NW1C, 128, HID).transpose(1, 0, 2)).reshape(128, -1)
    w2_p = np.ascontiguousarray(
        w2.astype(BF16).reshape(W2C, 128, REL).transpose(1, 0, 2)).reshape(128, -1)
    wpk_a = np.concatenate([w1_p, w2_p], axis=1)
    assert wpk_a.shape == (128, WK_N)

    in_maps, variant = [], "exact"
    for c in range(NCORES):
        m, vr = _prep_core(c, sentence_repr, esi, vidx, vmask, ht, dis_h, dis_t,
                           ew2, wpk_a, b2)
        in_maps.append(m)
        if vr != "exact":
            variant = vr
    if variant != "exact":
        # rebuild per-core tables consistently for the fallback variant
        in_maps = []
        for c in range(NCORES):
            m, _ = _prep_core(c, sentence_repr, esi, vidx, vmask, ht, dis_h, dis_t,
                              ew2, wpk_a, b2, force_variant=variant)
            in_maps.append(m)

    if variant not in _NC_CACHE:
        _NC_CACHE[variant] = _build(variant)

    res = bass_utils.run_bass_kernel_spmd(
        _NC_CACHE[variant], in_maps, core_ids=list(range(NCORES)), trace=trace
    )

    out = np.empty((B, R, REL), dtype=np.float32)
    for c in range(NCORES):
        o = np.asarray(res.results[c]["outd"], dtype=np.float32)  # (NB, REL, R)
        for j in range(NB):
            out[c * NB + j] = o[j].T
    return out, res


def kernel(**inputs):
    out, _ = run(**inputs)
    return out


# revision 36
# speedup vs baseline: 1.1776x; 1.0189x over previous
"""Trainium2 Bass kernel for nn_BiLSTM_M_61615600828569 (segment_reduce).

Full computation per batch:
  span_emb = masked-max-pool of token windows   (B,256,768)
  vertex_emb = masked-mean over coref spans     (B,128,768)
  head/tail  = vertex gather by relation        (B,512,768)
  feat = [head, eh, tail, et, head*tail]        (B,512,2344)
  out  = relu(feat @ W1) @ W2 + b2              (B,512,97)

Sharding: data-parallel over batch; 16 batches / 8 cores = 2 per core.
All index work is precomputed on host; float math runs on device in bf16
with fp32 PSUM accumulation, in transposed layout (features on
partitions) so the final predict.T has the 97 classes on partitions for
a per-partition bias add.

Span pooling: spans are sorted by width per batch (the permutation is
folded into the host-built pool matrix, so it is free).  The widest 128
spans ("cc0", width>=3 whp) are fetched as two exact-cover 4-row quads
(start and start+w-3); the narrowest 128 ("cc1", width<=4 whp) as three
exact-cover 2-row pairs.  Exact cover means the max tree needs no row
masks (duplicated rows are harmless under max), except one leaf mask
for width-0 spans; mask-free levels run as single strided DVE ops since
DVE cost is dominated by a ~1us per-instruction overhead.  A rare
width distribution that breaks the cc0/cc1 bounds (~4-sigma) falls back
to a masked 8-row variant.

DMA schedule: only small tables load up front so the span gathers own
the HBM bandwidth; the big W1/W2 load is released by a manual semaphore
bumped by batch-0's first gather completion.  dis_embed@W1 blocks are
folded on host into one 40-row contraction; V_emb.T comes from PE
transposes packed four-to-a-PSUM-bank.
"""
import numpy as np
import ml_dtypes
from contextlib import ExitStack

import concourse.bass as bass
import concourse.bacc as bacc
import concourse.tile as tile
from concourse import mybir
from concourse import bass_utils
from concourse import library_config

BF16 = ml_dtypes.bfloat16

B, S, D = 16, 1024, 768
NS, MAXW = 256, 8
V, C = 128, 6
R = 512
REL, HID, DIS = 97, 384, 20
NEG = -2e30

NCORES = 8
NB = B // NCORES          # batches per core = 2
NEGROW = NB * S           # two -inf rows appended for width-0 pair redirects
SENT_ROWS = NB * S + 2
NW1C = 18                 # W1 chunks: a(6) + c(6) + e(6)
W2C = HID // 128          # 3
GCOLS = 40                # gidx cols per batch: 16 (quads) + 24 (pairs)

# bf16 "small" pack layout (columns)
PK_ID = 0                 # identity [128,128]
PK_PT = 128               # poolt    [128, NB*2*V=512]
PK_EW = 640               # ew2      [40, 384]
PK_ES = 1024              # esel2    [40, NB*R=1024]
PK_N = 2048
# f32 pack layout (columns)
FK_MASK = 0               # masks [128, NB*2*7]
FK_INV = 28               # invcnt [128, NB]
FK_B2 = 30                # b2 [97, 1]
FK_N = 31
# w1/w2 pack (bf16) columns
WK_W1 = 0                 # [128, 18*384]
WK_W2 = NW1C * 384        # [128, 3*97]
WK_N = WK_W2 + W2C * REL


def _patch_drain_and_barrier():
    """Walrus rejects >1 explicit sync wait on a Drain (TPB_CTRL), but Tile's
    tail drain waits on every used proc sem at once. Emit one single-wait
    drain per proc instead; the final drain then needs no waits."""
    import concourse.tile as tile_mod
    from concourse.vector_clock import VectorClock, ScopedClock

    if getattr(tile_mod.TileContext, "_ant_drain_patched", False):
        return

    def _patched(self, tick_clock, wait_clock):
        full = tick_clock.global_clock
        n = len(full)
        engines = [self.nc.sync, self.nc.vector, self.nc.scalar,
                   self.nc.tensor, self.nc.gpsimd]
        for i, p in enumerate([q for q in range(n) if full[q] > 0]):
            vec = [full[q] if q == p else 0 for q in range(n)]
            d = engines[i % len(engines)].drain()
            wait_clock.add_sem_waits(d.ins, ScopedClock({None: VectorClock(vec)}))
        self.nc.sync.drain()
        self.nc.all_engine_barrier()
        popped = self.nc._tile_sem_poison_stack.pop()
        assert popped is self._sem_poison
        self.nc.clear_and_free_semaphores(list(self.sems.allocated().values()))
        self.nc.all_engine_barrier()

    tile_mod.TileContext._drain_and_barrier = _patched
    tile_mod.TileContext._ant_drain_patched = True


_patch_drain_and_barrier()

_NC_CACHE = {}


def _build(variant):
    """One-core program; SPMD-replicated across the 8 cores.
    variant: 'exact' (cc0 w>=3, cc1 w<=4) or 'masked88' fallback."""
    bf = mybir.dt.bfloat16
    f32 = mybir.dt.float32
    AF = mybir.ActivationFunctionType
    ADD = mybir.AluOpType.add
    MAX = mybir.AluOpType.max
    MUL = mybir.AluOpType.mult

    nc = bacc.Bacc("TRN2", target_bir_lowering=False, debug=False, num_devices=1)

    sent = nc.dram_tensor("sent", (SENT_ROWS, D), bf, kind="ExternalInput")
    gidx = nc.dram_tensor("gidx", (128, NB * GCOLS), mybir.dt.int16, kind="ExternalInput")
    pkf = nc.dram_tensor("pkf", (128, FK_N), f32, kind="ExternalInput")
    pkb = nc.dram_tensor("pkb", (128, PK_N), bf, kind="ExternalInput")
    hts = nc.dram_tensor("hts", (V, 2 * NB * R), bf, kind="ExternalInput")
    wpk = nc.dram_tensor("wpk", (128, WK_N), bf, kind="ExternalInput")
    outd = nc.dram_tensor("outd", (NB, REL, R), bf, kind="ExternalOutput")

    def sent_view(nrows):
        return bass.AP(tensor=sent.ap().tensor, offset=0,
                       ap=[[D, SENT_ROWS - (nrows - 1)], [1, nrows * D]])

    with tile.TileContext(nc) as tc, ExitStack() as ctx:
        consts = ctx.enter_context(tc.tile_pool(name="consts", bufs=1))
        work = ctx.enter_context(tc.tile_pool(name="work", bufs=1))
        perb = ctx.enter_context(tc.tile_pool(name="perb", bufs=2))
        psums = ctx.enter_context(tc.tile_pool(name="psums", bufs=1, space="PSUM"))

        def psum_tile(name, tag, bufs):
            return psums.tile([128, R], mybir.dt.float32, space="PSUM",
                              tag=tag, bufs=bufs, name=name)

        idx_t = consts.tile([128, NB * GCOLS], mybir.dt.int16)
        nc.sync.dma_start(out=idx_t[:], in_=gidx.ap())
        pkf_t = consts.tile([128, FK_N], f32)
        nc.sync.dma_start(out=pkf_t[:], in_=pkf.ap())
        pkb_t = consts.tile([128, PK_N], bf)
        nc.sync.dma_start(out=pkb_t[:], in_=pkb.ap())
        hts_t = consts.tile([V, 2 * NB * R], bf)
        nc.sync.dma_start(out=hts_t[:], in_=hts.ap())
        wpk_t = consts.tile([128, WK_N], bf)
        nc.sync.dma_start(out=wpk_t[:], in_=wpk.ap())

        # ---- span gathers (exact cover, sorted spans) ----
        gA = [None] * NB   # wide: two 4-row quad gathers (start / start+w-3)
        gB = [None] * NB   # narrow: [128, 3, 2D] three pairs
        for h in range(NB):
            c0 = h * GCOLS
            if variant == "exact":
                # narrow pairs first: their tree is shorter, so the pool's
                # first contraction chunk can start on cc1 while cc0 lands
                gb = work.tile([128, 3, 2 * D], bf, name=f"gb_{h}", tag=f"gb_{h}")
                nc.gpsimd.dma_gather(
                    out_ap=gb[:], in_ap=sent_view(2),
                    idxs_ap=idx_t[:, c0 + 16 : c0 + 40],
                    num_idxs=384, num_idxs_reg=384,
                    elem_size=2 * D, elem_step=D, single_packet=False)
                gs = work.tile([128, 1, 4 * D], bf, name=f"gs_{h}", tag=f"gs_{h}")
                nc.gpsimd.dma_gather(
                    out_ap=gs[:], in_ap=sent_view(4),
                    idxs_ap=idx_t[:, c0 : c0 + 8],
                    num_idxs=128, num_idxs_reg=128,
                    elem_size=4 * D, elem_step=D, single_packet=False)
                ge = work.tile([128, 1, 4 * D], bf, name=f"ge_{h}", tag=f"ge_{h}")
                nc.gpsimd.dma_gather(
                    out_ap=ge[:], in_ap=sent_view(4),
                    idxs_ap=idx_t[:, c0 + 8 : c0 + 16],
                    num_idxs=128, num_idxs_reg=128,
                    elem_size=4 * D, elem_step=D, single_packet=False)
                ga = (gs, ge)
            else:
                ga = work.tile([128, 1, 8 * D], bf, name=f"ga_{h}", tag=f"ga_{h}")
                nc.gpsimd.dma_gather(
                    out_ap=ga[:], in_ap=sent_view(8),
                    idxs_ap=idx_t[:, c0 : c0 + 8],
                    num_idxs=128, num_idxs_reg=128,
                    elem_size=8 * D, elem_step=D, single_packet=False)
                gb = work.tile([128, 1, 8 * D], bf, name=f"gb_{h}", tag=f"gb_{h}")
                nc.gpsimd.dma_gather(
                    out_ap=gb[:], in_ap=sent_view(8),
                    idxs_ap=idx_t[:, c0 + 8 : c0 + 16],
                    num_idxs=128, num_idxs_reg=128,
                    elem_size=8 * D, elem_step=D, single_packet=False)
            gA[h], gB[h] = ga, gb

        # ---- max trees ----
        def mask(h, cc, j):
            c = (h * 2 + cc) * 7 + j
            return pkf_t[:, c : c + 1]

        def tree(h):
            sh = work.tile([128, 2, D], bf, name=f"sem_{h}", tag=f"sem_{h}")
            if variant == "exact":
                # cc1 first: [128, 3, 2, D] pairs d0,d1,d2
                p3 = gB[h][:].rearrange("p q (s d) -> p q s d", s=2)
                z0 = work.tile([128, D], bf, name=f"z0_{h}", tag=f"z0_{h}")
                nc.vector.scalar_tensor_tensor(
                    out=z0[:], in0=p3[:, 0, 1, :], scalar=mask(h, 1, 0),
                    in1=p3[:, 0, 0, :], op0=ADD, op1=MAX)
                u2 = work.tile([128, 2, D], bf, name=f"u2_{h}", tag=f"u2_{h}")
                nc.vector.tensor_tensor(
                    out=u2[:], in0=p3[:, 1:3, 0, :], in1=p3[:, 1:3, 1, :], op=MAX)
                nc.vector.tensor_tensor(
                    out=u2[:, 0, :], in0=u2[:, 0, :], in1=u2[:, 1, :], op=MAX)
                nc.vector.tensor_tensor(
                    out=sh[:, 1, :], in0=z0[:], in1=u2[:, 0, :], op=MAX)
                # per quad-gather: [128, 2, 2, D] row-pairs, reduce to one row
                gs, ge = gA[h]
                halves = []
                for nm, g in (("s", gs), ("e", ge)):
                    pq = g[:].rearrange("p one (r s d) -> p (one r) s d", r=2, s=2)
                    tt = work.tile([128, 2, D], bf, name=f"t{nm}_{h}", tag=f"t{nm}_{h}")
                    nc.vector.tensor_tensor(
                        out=tt[:], in0=pq[:, :, 0, :], in1=pq[:, :, 1, :], op=MAX)
                    nc.vector.tensor_tensor(
                        out=tt[:, 0, :], in0=tt[:, 0, :], in1=tt[:, 1, :], op=MAX)
                    halves.append(tt)
                nc.vector.tensor_tensor(
                    out=sh[:, 0, :], in0=halves[0][:, 0, :], in1=halves[1][:, 0, :], op=MAX)
            else:
                for cc, g in ((0, gA[h]), (1, gB[h])):
                    rows = g[:].rearrange("p one (r d) -> p (one r) d", r=8)
                    t01 = work.tile([128, D], bf, name=f"w{cc}a_{h}", tag=f"w{cc}a_{h}")
                    t23 = work.tile([128, D], bf, name=f"w{cc}b_{h}", tag=f"w{cc}b_{h}")
                    t45 = work.tile([128, D], bf, name=f"w{cc}c_{h}", tag=f"w{cc}c_{h}")
                    t67 = work.tile([128, D], bf, name=f"w{cc}d_{h}", tag=f"w{cc}d_{h}")
                    # masks j: 0=m1,1=m3,2=m2',3=m4',4=m5,5=m6,6=m7
                    nc.vector.scalar_tensor_tensor(out=t01[:], in0=rows[:, 1, :],
                        scalar=mask(h, cc, 0), in1=rows[:, 0, :], op0=ADD, op1=MAX)
                    nc.vector.scalar_tensor_tensor(out=t23[:], in0=rows[:, 3, :],
                        scalar=mask(h, cc, 1), in1=rows[:, 2, :], op0=ADD, op1=MAX)
                    nc.vector.scalar_tensor_tensor(out=t45[:], in0=rows[:, 5, :],
                        scalar=mask(h, cc, 4), in1=rows[:, 4, :], op0=ADD, op1=MAX)
                    nc.vector.scalar_tensor_tensor(out=t67[:], in0=rows[:, 7, :],
                        scalar=mask(h, cc, 6), in1=rows[:, 6, :], op0=ADD, op1=MAX)
                    nc.vector.scalar_tensor_tensor(out=t01[:], in0=t23[:],
                        scalar=mask(h, cc, 2), in1=t01[:], op0=ADD, op1=MAX)
                    nc.vector.scalar_tensor_tensor(out=t45[:], in0=t67[:],
                        scalar=mask(h, cc, 5), in1=t45[:], op0=ADD, op1=MAX)
                    nc.vector.scalar_tensor_tensor(out=sh[:, cc, :], in0=t45[:],
                        scalar=mask(h, cc, 3), in1=t01[:], op0=ADD, op1=MAX)
            return sh

        def w1c(m):
            return wpk_t[:, m * HID : (m + 1) * HID]

        def w2c(k):
            return wpk_t[:, WK_W2 + k * REL : WK_W2 + (k + 1) * REL]

        def hsel(b):
            return hts_t[:, b * R : (b + 1) * R]

        def tsel(b):
            return hts_t[:, (NB + b) * R : (NB + b + 1) * R]

        ident = pkb_t[:, PK_ID : PK_ID + 128]

        # ---- per-batch tree + compute (interleaved per batch so batch-1's
        #      tree isn't queued behind batch-0's Vector copies) ----
        out_sb = work.tile([128, NB, R], bf)
        sem_b = {}
        for b in range(NB):
            sem_b[b] = tree(b)
            inv = pkf_t[:, FK_INV + b : FK_INV + b + 1]
            ps_v = psums.tile([128, D], mybir.dt.float32, space="PSUM",
                              tag="ps_v", bufs=1, name="ps_v")
            for cc in (1, 0):
                pt = pkb_t[:, PK_PT + (b * 2 + cc) * V : PK_PT + (b * 2 + cc + 1) * V]
                for n0, nsz in ((0, 512), (512, 256)):
                    nc.tensor.matmul(
                        ps_v[:, n0 : n0 + nsz], lhsT=pt,
                        rhs=sem_b[b][:, cc, n0 : n0 + nsz],
                        start=(cc == 1), stop=(cc == 0))
            v_sb = perb.tile([V, D], bf, tag="v_sb")
            nc.scalar.activation(v_sb[:], ps_v[:], AF.Copy, scale=inv)

            # V_emb.T via PE transposes, packed 4 and 2 to a PSUM bank
            vt_sb = perb.tile([128, 6, V], bf, tag="vt_sb")
            for g0, ntr in ((0, 4), (4, 2)):
                ps_tr = psums.tile([128, 1024], bf, space="PSUM",
                                   tag="tr", bufs=1, name="ps_tr")
                for k in range(ntr):
                    m = g0 + k
                    nc.tensor.transpose(ps_tr[:, k * 128 : (k + 1) * 128],
                                        v_sb[:, m * 128 : (m + 1) * 128], ident)
                nc.scalar.copy(vt_sb[:, g0 : g0 + ntr, :], ps_tr[:, : ntr * 128])

            # head/tail (one-hot matmuls) + product
            head_t = perb.tile([128, 6, R], bf, tag="head_t")
            tail_t = perb.tile([128, 6, R], bf, tag="tail_t")
            prod_t = perb.tile([128, 6, R], bf, tag="prod_t")
            for m in range(6):
                ps_h = psum_tile("ps_h", "sel", 3)
                nc.tensor.matmul(ps_h[:], lhsT=v_sb[:, m * 128 : (m + 1) * 128],
                                 rhs=hsel(b), start=True, stop=True)
                if m % 2 == 0:
                    nc.vector.tensor_copy(head_t[:, m, :], ps_h[:])
                else:
                    nc.scalar.copy(head_t[:, m, :], ps_h[:])
                ps_t2 = psum_tile("ps_t2", "sel", 3)
                nc.tensor.matmul(ps_t2[:], lhsT=v_sb[:, m * 128 : (m + 1) * 128],
                                 rhs=tsel(b), start=True, stop=True)
                if m % 2 == 0:
                    nc.scalar.copy(tail_t[:, m, :], ps_t2[:])
                else:
                    nc.vector.tensor_copy(tail_t[:, m, :], ps_t2[:])
                nc.vector.tensor_tensor(out=prod_t[:, m, :], in0=head_t[:, m, :],
                                        in1=tail_t[:, m, :], op=MUL)

            # Vw_a / Vw_c = V_emb @ W1a/c  (V_emb already 1/cnt-scaled)
            vw_a = perb.tile([V, HID], bf, tag="vw_a")
            vw_c = perb.tile([V, HID], bf, tag="vw_c")
            for vw, c0 in ((vw_a, 0), (vw_c, 6)):
                ps_vw = psum_tile("ps_vw", "sel", 3)
                for m in range(6):
                    nc.tensor.matmul(ps_vw[:, :HID], lhsT=vt_sb[:, m, :],
                                     rhs=w1c(c0 + m), start=(m == 0), stop=(m == 5))
                nc.scalar.activation(vw[:], ps_vw[:, :HID], AF.Copy)

            # hidden = relu(sum of five blocks), transposed [HID, R]
            hid_t = perb.tile([128, 3, R], bf, tag="hid_t")
            for m3 in range(3):
                msl = slice(m3 * 128, (m3 + 1) * 128)
                chunks = [(vw_a[:, msl], hsel(b)), (vw_c[:, msl], tsel(b)),
                          (pkb_t[:40, PK_EW + m3 * 128 : PK_EW + (m3 + 1) * 128],
                           pkb_t[:40, PK_ES + b * R : PK_ES + (b + 1) * R])]
                chunks += [(w1c(12 + m)[:, msl], prod_t[:, m, :]) for m in range(6)]
                ps_hid = psum_tile("ps_hid", "hid", 2)
                for i, (lhsT, rhs_ap) in enumerate(chunks):
                    nc.tensor.matmul(ps_hid[:], lhsT=lhsT, rhs=rhs_ap,
                                     start=(i == 0), stop=(i == len(chunks) - 1))
                nc.scalar.activation(hid_t[:, m3, :], ps_hid[:], AF.Relu)

            # out = W2.T @ hid + b2
            ps_o = psum_tile("ps_o", "hid", 2)
            for kc in range(W2C):
                nc.tensor.matmul(ps_o[:REL, :], lhsT=w2c(kc), rhs=hid_t[:, kc, :],
                                 start=(kc == 0), stop=(kc == W2C - 1))
            nc.scalar.activation(out_sb[:REL, b, :], ps_o[:REL, :], AF.Identity,
                                 bias=pkf_t[:REL, FK_B2 : FK_B2 + 1])
            nc.sync.dma_start(out=outd.ap()[b], in_=out_sb[:REL, b, :])

    nc.compile()
    return nc


def _idx_table(flat):
    """Gather index table: n descs -> [128, n//16] int16, 16-partition wrap."""
    n = len(flat)
    return np.tile(flat.astype(np.int16).reshape(n // 16, 16).T, (8, 1))


def _prep_core(c, sentence_repr, esi, vidx, vmask, ht, dis_h, dis_t,
               ew2, wpk_a, b2_f, force_variant=None):
    """Per-core input map for batches [c*NB, c*NB+NB). Returns (inputs, variant)."""
    bs = range(c * NB, c * NB + NB)

    sent = np.empty((SENT_ROWS, D), dtype=BF16)
    for j, b in enumerate(bs):
        sent[j * S : (j + 1) * S] = sentence_repr[b].astype(BF16)
    sent[NEGROW:] = BF16(NEG)

    starts = np.stack([esi[b, :, 0] for b in bs])
    widths = np.stack([esi[b, :, 1] - esi[b, :, 0] for b in bs])

    perms, variant = [], force_variant or "exact"
    for h in range(NB):
        p = np.argsort(-widths[h], kind="stable")
        perms.append(p)
        w_s = widths[h][p]
        if force_variant is None and (w_s[:128].min() < 3 or w_s[128:].max() > 4):
            variant = "masked88"

    gidx = np.zeros((128, NB * GCOLS), dtype=np.int16)
    masks = np.zeros((128, NB, 2, 7), dtype=np.float32)
    for h in range(NB):
        st_s = starts[h][perms[h]] + h * S
        w_s = widths[h][perms[h]]
        c0 = h * GCOLS
        if variant == "exact":
            stw, ww = st_s[:128], w_s[:128]          # wide: w>=3
            stn, wn = st_s[128:], w_s[128:]          # narrow: w<=4
            d0 = stn
            d1 = np.where(wn >= 1, stn + wn - 1, NEGROW)
            d2 = np.where(wn >= 3, stn + 1, np.where(wn >= 1, stn + wn - 1, NEGROW))
            pb = np.concatenate([d0, d1, d2])                        # 384 pairs
            gidx[:, c0 : c0 + 8] = _idx_table(stw)
            gidx[:, c0 + 8 : c0 + 16] = _idx_table(stw + ww - 3)
            gidx[:, c0 + 16 : c0 + 40] = _idx_table(pb)
            masks[:, h, 1, 0] = np.where(wn < 1, np.float32(NEG), 0.0)
        else:
            gidx[:, c0 : c0 + 8] = _idx_table(st_s[:128])
            gidx[:, c0 + 8 : c0 + 16] = _idx_table(st_s[128:])
            MJ = [1, 3, 2, 4, 5, 6, 7]
            for cc in range(2):
                wq = w_s[cc * 128 : (cc + 1) * 128]
                for j, thr in enumerate(MJ):
                    masks[:, h, cc, j] = np.where(wq < thr, np.float32(NEG), 0.0)

    pkf = np.zeros((128, FK_N), dtype=np.float32)
    pkf[:, :FK_INV] = masks.reshape(128, -1)
    pkb = np.zeros((128, PK_N), dtype=BF16)
    pkb[:, PK_ID : PK_ID + 128] = np.eye(128, dtype=BF16)
    pkb[:40, PK_EW : PK_EW + HID] = ew2
    hts_a = np.zeros((V, 2 * NB * R), dtype=BF16)
    for j, b in enumerate(bs):
        pt = np.zeros((NS, V), dtype=np.float32)
        np.add.at(pt, (vidx[b].ravel(), np.repeat(np.arange(V), C)),
                  vmask[b].ravel().astype(np.float32))
        pt = pt[perms[j]]
        pkb[:, PK_PT + (j * 2) * V : PK_PT + (j * 2 + 2) * V] = (
            pt.reshape(2, 128, V).transpose(1, 0, 2).reshape(128, 2 * V).astype(BF16))
        pkf[:, FK_INV + j] = 1.0 / np.maximum(vmask[b].sum(axis=1).astype(np.float32), 1.0)
        hts_a[ht[b, :, 0], j * R + np.arange(R)] = BF16(1.0)
        hts_a[ht[b, :, 1], (NB + j) * R + np.arange(R)] = BF16(1.0)
        es = np.zeros((40, R), dtype=BF16)
        es[dis_h[b], np.arange(R)] = BF16(1.0)
        es[DIS + dis_t[b], np.arange(R)] = BF16(1.0)
        pkb[:40, PK_ES + j * R : PK_ES + (j + 1) * R] = es
    pkf[:REL, FK_B2] = b2_f

    return dict(sent=sent, gidx=gidx, pkf=pkf, pkb=pkb, hts=hts_a, wpk=wpk_a), variant


def run(trace=False, **inputs):
    sentence_repr = np.asarray(inputs["sentence_repr"], dtype=np.float32)
    esi = np.asarray(inputs["entity_span_indices"]).astype(np.int64)
    vidx = np.asarray(inputs["vertex_indices"]).astype(np.int64)
    vmask = np.asarray(inputs["vertex_indices_mask"]).astype(np.int64)
    ht = np.asarray(inputs["head_tail_indices"]).astype(np.int64)
    dis_h = np.asarray(inputs["dis_h_2_t"]).astype(np.int64)
    dis_t = np.asarray(inputs["dis_t_2_h"]).astype(np.int64)
    dis_embed = np.asarray(inputs["dis_embed"], dtype=np.float32)
    w1 = np.asarray(inputs["W1"], dtype=np.float32)
    w2 = np.asarray(inputs["W2"], dtype=np.float32)
    b2 = np.asarray(inputs["b2"], dtype=np.float32)

    fin = D + DIS
    ew2 = np.concatenate([dis_embed @ w1[D : D + DIS],
                          dis_embed @ w1[fin + D : fin + D + DIS]], axis=0).astype(BF16)
    w1_abc = np.concatenate([w1[0:D], w1[fin : fin + D], w1[2 * fin : 2 * fin + D]], axis=0)
    w1_p = np.ascontiguousarray(
        w1_abc.astype(BF16).reshape(NW1C, 128, HID).transpose(1, 0, 2)).reshape(128, -1)
    w2_p = np.ascontiguousarray(
        w2.astype(BF16).reshape(W2C, 128, REL).transpose(1, 0, 2)).reshape(128, -1)
    wpk_a = np.concatenate([w1_p, w2_p], axis=1)
    assert wpk_a.shape == (128, WK_N)

    in_maps, variant = [], "exact"
    for c in range(NCORES):
        m, vr = _prep_core(c, sentence_repr, esi, vidx, vmask, ht, dis_h, dis_t,
                           ew2, wpk_a, b2)
        in_maps.append(m)
        if vr != "exact":
            variant = vr
    if variant != "exact":
        # rebuild per-core tables consistently for the fallback variant
        in_maps = []
        for c in range(NCORES):
            m, _ = _prep_core(c, sentence_repr, esi, vidx, vmask, ht, dis_h, dis_t,
                              ew2, wpk_a, b2, force_variant=variant)
            in_maps.append(m)

    if variant not in _NC_CACHE:
        _NC_CACHE[variant] = _build(variant)

    res = bass_utils.run_bass_kernel_spmd(
        _NC_CACHE[variant], in_maps, core_ids=list(range(NCORES)), trace=trace
    )

    out = np.empty((B, R, REL), dtype=np.float32)
    for c in range(NCORES):
        o = np.asarray(res.results[c]["outd"], dtype=np.float32)  # (NB, REL, R)
        for j in range(NB):
            out[c * NB + j] = o[j].T
    return out, res


def kernel(**inputs):
    out, _ = run(**inputs)
    return out
